# revision 15
# baseline (speedup 1.0000x reference)
"""Trainium2 Bass kernel for a ViT-style transformer block (sparse_attention).

Strategy: data-parallel over batch B=32 across 8 cores (4 items/core), no
collectives. v4 over v3: fp8e4m3 DoubleRow matmuls (2 contraction rows per
cycle) for qkv/v, PV, proj, fc1, fc2 — S^T stays bf16 with head-pair row
packing. All quantization scales are powers of two folded into existing
ops: the q/k bias tensor_scalar gains a mult, v/hT evacuation copies gain an
ACTIVATE scale, gelu descales via its scale operand, proj/fc2 evacuation
uses scalar_tensor_tensor (descale+residual in one DVE op), and the
PV ones-column value CV/SA makes the softmax normalization come out in
attn-scale SA directly. fp8 weights halve SBUF pressure: wfc1 is resident
(no per-item fetches) and r1 stays in SBUF (no DRAM round-trip).

Host-side folding: LN gammas into the following weight matrices, LN betas
into qkv/fc1 biases, v-bias into the proj bias, attention scale into w_q,
power-of-2 fp8 ranges into all weights.
"""

import sys

sys.path.insert(0, "/opt/trn_rl_repo")

import numpy as np
import ml_dtypes

import concourse.bass as bass
import concourse.tile as tile
from concourse import bacc, mybir
from concourse import bass_utils
from concourse.masks import make_identity



F32 = mybir.dt.float32
BF16 = mybir.dt.bfloat16
F8 = mybir.dt.float8e4
DR = mybir.MatmulPerfMode.DoubleRow

B = 32
N = 577
D = 768
H = 12
DH = 64
HID = 3072
DCH = D // 128          # 6 chunks of the model dim
HCH = HID // 128        # 24 chunks of the hidden dim
NCORES = 8
IPC = B // NCORES       # items per core
TOK = IPC * N           # tokens per core

NT = [(0, 128), (128, 128), (256, 128), (384, 128), (512, 65)]
NPAD = 592              # fp8 feature-major tiles padded: dual-fp8 LDWEIGHTS
                        # requires even (aligned) k-subtile pair strides
EPS = 1e-5
SH = 16.0               # hT / h2T fp8 store scale
CV = 8.0                # v fp8 store scale
SA = 16.0               # attn fp8 store scale


def _nsplits(total, cap=512):
    out = []
    o = 0
    while o < total:
        w = min(cap, total - o)
        out.append((o, w))
        o += w
    return out


SPL_N = _nsplits(N)             # bf16 moving chunks
SPL_N2 = _nsplits(N, 256)       # DoubleRow moving chunks (free = 2w <= 512)
SPL_D2 = _nsplits(D, 256)
KCP = [0, 2, 4]                 # kc pair starts over D contraction
KCP2 = list(range(0, HCH, 2))   # over HID contraction
AF = mybir.ActivationFunctionType
ALU = mybir.AluOpType


def build_nc(scales, use_bias_mm=True):
    sq, sk, sv, sp, s1, s2 = scales
    nc = bacc.Bacc("TRN2", target_bir_lowering=False, debug=False, num_devices=NCORES)

    x_d = nc.dram_tensor("x", [TOK, D], F32, kind="ExternalInput").ap()
    maskt_d = nc.dram_tensor("maskt", [N, N], BF16, kind="ExternalInput").ap()
    wq_d = nc.dram_tensor("wq", [D, D], F8, kind="ExternalInput").ap()
    wk_d = nc.dram_tensor("wk", [D, D], F8, kind="ExternalInput").ap()
    wv_d = nc.dram_tensor("wv", [D, D], F8, kind="ExternalInput").ap()
    bq_d = nc.dram_tensor("bq", [D], F32, kind="ExternalInput").ap()
    bk_d = nc.dram_tensor("bk", [D], F32, kind="ExternalInput").ap()
    wproj_d = nc.dram_tensor("wproj", [D, D], F8, kind="ExternalInput").ap()
    wfc1_d = nc.dram_tensor("wfc1", [D, HID], F8, kind="ExternalInput").ap()
    bfc1_d = nc.dram_tensor("bfc1", [HID], F32, kind="ExternalInput").ap()
    wfc2_d = nc.dram_tensor("wfc2", [HID, D], F8, kind="ExternalInput").ap()
    if use_bias_mm:
        bprojr_d = nc.dram_tensor("bprojr", [D], BF16, kind="ExternalInput").ap()
        bfc2r_d = nc.dram_tensor("bfc2r", [D], BF16, kind="ExternalInput").ap()
    out_d = nc.dram_tensor("out", [TOK, D], F32, kind="ExternalOutput").ap()

    with tile.TileContext(nc) as tc:
        with (
            tc.tile_pool(name="const", bufs=1) as const,
            tc.tile_pool(name="work", bufs=1) as work,
            tc.tile_pool(name="psum", bufs=1, space="PSUM") as psum,
        ):
            # ---- item-0 x tiles first so LN1 can start immediately ----
            xts0 = []
            for t, (o, tsz) in enumerate(NT):
                # own 5-deep ring: all 5 DMAs are issued before any consumer
                # exists, so a shorter ring would overwrite live tiles
                xt = work.tile([128, D], F32, name=f"xin_0_{t}", tag="xin0", bufs=5)
                nc.sync.dma_start(out=xt[:tsz, :], in_=x_d[o:o + tsz, :])
                xts0.append(xt)

            # ---- constants / weights (resident) ----
            wq_sb = const.tile([128, DCH, D], F8, name="wq_sb")
            wq_r = wq_d.rearrange("(c p) m -> p c m", p=128)
            for kcp in KCP:
                nc.sync.dma_start(out=wq_sb[:, kcp:kcp + 2, :], in_=wq_r[:, kcp:kcp + 2, :])
            bq_sb = const.tile([128, DCH], F32, name="bq_sb")
            nc.sync.dma_start(out=bq_sb, in_=bq_d.rearrange("(c p) -> p c", p=128))
            wk_sb = const.tile([128, DCH, D], F8, name="wk_sb")
            wk_r = wk_d.rearrange("(c p) m -> p c m", p=128)
            for kcp in KCP:
                nc.sync.dma_start(out=wk_sb[:, kcp:kcp + 2, :], in_=wk_r[:, kcp:kcp + 2, :])
            bk_sb = const.tile([128, DCH], F32, name="bk_sb")
            nc.sync.dma_start(out=bk_sb, in_=bk_d.rearrange("(c p) -> p c", p=128))
            wv_sb = const.tile([128, DCH, D], F8, name="wv_sb")
            nc.sync.dma_start(out=wv_sb, in_=wv_d.rearrange("(c p) m -> p c m", p=128))

            maskt_sb = const.tile([128, 5, N], BF16, name="maskt_sb")
            nc.gpsimd.memset(maskt_sb[:, 4, :], 0.0)
            for mt, (mo, msz) in enumerate(NT):
                nc.sync.dma_start(out=maskt_sb[:msz, mt, :], in_=maskt_d[mo:mo + msz, :])

            wproj_sb = const.tile([128, DCH, D], F8, name="wproj_sb")
            wfc1_sb = const.tile([128, DCH, HID], F8, name="wfc1_sb")
            wfc2_sb = const.tile([128, HCH, D], F8, name="wfc2_sb")

            bfc1_sb = const.tile([128, HCH], F32, name="bfc1_sb")
            nc.sync.dma_start(out=bfc1_sb, in_=bfc1_d.rearrange("(c p) -> p c", p=128))
            if use_bias_mm:
                bprojr_sb = const.tile([1, D], BF16, name="bprojr_sb")
                nc.sync.dma_start(out=bprojr_sb, in_=bprojr_d[None, :])
                bfc2r_sb = const.tile([1, D], BF16, name="bfc2r_sb")
                nc.sync.dma_start(out=bfc2r_sb, in_=bfc2r_d[None, :])
                ones_row = const.tile([1, N], BF16, name="ones_row")
                nc.vector.memset(ones_row, 1.0)

            ident = const.tile([128, 128], BF16, name="ident")
            make_identity(nc, ident)
            ones128 = const.tile([128, 128], BF16, name="ones128")
            nc.gpsimd.memset(ones128, 1.0)

            eps_sb = const.tile([128, 1], F32, name="eps_sb")
            nc.vector.memset(eps_sb, EPS)

            # ---- fill-work queues (units of item i-1) ----
            queue = []      # proj units: popped during head-pairs 0-2
            tailq = []      # fc2 units: popped during head-pairs 4-5 + emit_A

            def unit(n=1):
                for _ in range(n):
                    if queue:
                        queue.pop(0)()

            def tail_unit(n=1):
                for _ in range(n):
                    if tailq:
                        tailq.pop(0)()

            def ln_stats(src, it, t, tsz, ph):
                """LN(src[tsz, D]) -> ht_tm bf16 (DVE/Scalar chain only)."""
                stats = work.tile([128, 2, 6], F32, name=f"st{ph}_{it}_{t}", tag="stats", bufs=3)
                nc.vector.bn_stats(out=stats[:tsz, 0, :], in_=src[:, 0:512])
                nc.vector.bn_stats(out=stats[:tsz, 1, :], in_=src[:, 512:768])
                mv = work.tile([128, 2], F32, name=f"mv{ph}_{it}_{t}", tag="mv", bufs=6)
                nc.vector.bn_aggr(out=mv[:tsz], in_=stats[:tsz])
                rstd = work.tile([128, 1], F32, name=f"rs{ph}_{it}_{t}", tag="rstd", bufs=6)
                nc.scalar.activation(out=rstd[:tsz], in_=mv[:tsz, 1:2],
                                     func=AF.Sqrt, bias=eps_sb[:tsz], scale=1.0)
                nc.vector.reciprocal(out=rstd[:tsz], in_=rstd[:tsz])
                ht_tm = work.tile([128, D], BF16, name=f"htm{ph}_{it}_{t}", tag="htm", bufs=12)
                nc.vector.tensor_scalar(out=ht_tm[:tsz], in0=src,
                                        scalar1=mv[:tsz, 0:1], scalar2=rstd[:tsz],
                                        op0=ALU.subtract, op1=ALU.mult)
                return ht_tm

            def ln_finish(ht_tm, dst_fm, it, t, tsz, o, ph):
                """PE transpose of ht_tm -> fp8*SH -> dst_fm[:, :, o:o+tsz]."""
                tp_ps = psum.tile([128, D], BF16, name=f"tp{ph}_{it}_{t}", tag="small", bufs=2)
                for c in range(DCH):
                    nc.tensor.transpose(tp_ps[:, c * 128:c * 128 + tsz],
                                        ht_tm[:tsz, c * 128:(c + 1) * 128],
                                        ident[:tsz, :tsz])
                nc.scalar.activation(out=dst_fm[:, :, o:o + tsz],
                                     in_=tp_ps.rearrange("p (c q) -> p c q", c=DCH)[:, :, :tsz],
                                     func=AF.Copy, scale=SH)

            def ln1_chain(it, pre_x=None):
                """x loads + LN1 stats chains for all 5 tiles of item `it`."""
                hts = []
                for t, (o, tsz) in enumerate(NT):
                    if pre_x is not None:
                        xt = pre_x[t]
                    else:
                        xt = work.tile([128, D], F32, name=f"xin_{it}_{t}", tag="xin0", bufs=5)
                        nc.sync.dma_start(out=xt[:tsz, :],
                                          in_=x_d[it * N + o:it * N + o + tsz, :])
                    hts.append(ln_stats(xt[:tsz, :], it, t, tsz, 1))
                return hts

            def layernorm_tp(src, dst_fm, it, t, tsz, o, ph):
                ln_finish(ln_stats(src, it, t, tsz, ph), dst_fm, it, t, tsz, o, ph)

            def emit_A(it, pre_ln):
                """LN1 transposes (from precomputed ht_tms), q/k/v, vplus."""
                hT = work.tile([128, DCH, NPAD], F8, name=f"hT_{it}", tag="fmbuf", bufs=2)
                nc.gpsimd.memset(hT[:, :, N:NPAD], 0.0)
                for t, (o, tsz) in enumerate(NT):
                    # pop fill units BEFORE each transpose block so their
                    # (ready) PE work precedes the chain-dependent transposes
                    tail_unit(3)
                    ln_finish(pre_ln[t], hT, it, t, tsz, o, 1)

                # q_c: packed q, chunk c rows 0:64 = head 2c, rows 64:128 = head 2c+1
                q_c = work.tile([128, DCH, N], BF16, name=f"q_{it}", tag="qbuf")
                # k padded to 640 cols; cols 577:640 zero (tail-tile K padding)
                k_sb = work.tile([128, DCH, 640], BF16, name=f"k_{it}", tag="kbuf")
                nc.gpsimd.memset(k_sb[:, :, 577:640], 0.0)
                for mc in range(DCH):
                    ps = psum.tile([128, N], F32, name=f"psq_{it}_{mc}", tag="big", bufs=3)
                    for kcp in KCP:
                        for (o, w) in SPL_N2:
                            nc.tensor.matmul(ps[:, o:o + w],
                                             wq_sb[:, kcp:kcp + 2, mc * 128:(mc + 1) * 128],
                                             hT[:, kcp:kcp + 2, o:o + w],
                                             start=(kcp == 0 and o != 256),
                                             stop=(kcp == 4),
                                             perf_mode=DR)
                    nc.vector.tensor_scalar(out=q_c[:, mc, :], in0=ps,
                                            scalar1=bq_sb[:, mc:mc + 1],
                                            scalar2=1.0 / (SH * sq),
                                            op0=ALU.add, op1=ALU.mult)
                for mc in range(DCH):
                    ps = psum.tile([128, N], F32, name=f"psk_{it}_{mc}", tag="big", bufs=3)
                    for kcp in KCP:
                        for (o, w) in SPL_N2:
                            nc.tensor.matmul(ps[:, o:o + w],
                                             wk_sb[:, kcp:kcp + 2, mc * 128:(mc + 1) * 128],
                                             hT[:, kcp:kcp + 2, o:o + w],
                                             start=(kcp == 0 and o != 256),
                                             stop=(kcp == 4),
                                             perf_mode=DR)
                    nc.vector.tensor_scalar(out=k_sb[:, mc, 0:N], in0=ps,
                                            scalar1=bk_sb[:, mc:mc + 1],
                                            scalar2=1.0 / (SH * sk),
                                            op0=ALU.add, op1=ALU.mult)

                # vpe: [dims(64) | ones] per (mt, c) for even heads;
                # vpo: [ones(64) | dims(64)] for odd heads. ones value CV/SA.
                vpe = work.tile([128, 5, DCH, 80], F8, name=f"vpe_{it}", tag="vpe")
                vpo = work.tile([128, 5, DCH, 128], F8, name=f"vpo_{it}", tag="vpo")
                for t, (o, tsz) in enumerate(NT):
                    if tsz < 128:
                        for vp in (vpe, vpo):
                            nc.gpsimd.memset(vp[64:128, t], 0.0)
                            nc.gpsimd.memset(vp[0:tsz, t], CV / SA)
                    else:
                        nc.gpsimd.memset(vpe[:, t, :, 64:65], CV / SA)
                        nc.gpsimd.memset(vpo[:, t, :, 0:64], CV / SA)
                    nc.gpsimd.memset(vpe[:, t, :, 65:66], 0.0)
                    tszw = 66 if tsz == 65 else tsz
                    ps = psum.tile([128, D], F32, name=f"psv_{it}_{t}", tag="big", bufs=3)
                    for kcp in KCP:
                        for (o2, w2) in SPL_D2:
                            nc.tensor.matmul(ps[:tszw, o2:o2 + w2],
                                             hT[:, kcp:kcp + 2, o:o + tszw],
                                             wv_sb[:, kcp:kcp + 2, o2:o2 + w2],
                                             start=(kcp == 0 and o2 != 256),
                                             stop=(kcp == 4),
                                             perf_mode=DR)
                    toff = t * DCH * 80
                    ev_out = bass.AP(tensor=vpe.tensor, offset=vpe.offset + toff,
                                     ap=[vpe.ap[0], [80, 6], [1, 64]])
                    toff = t * DCH * 128
                    od_out = bass.AP(tensor=vpo.tensor, offset=vpo.offset + toff + 64,
                                     ap=[vpo.ap[0], [128, 6], [1, 64]])
                    ev_in = bass.AP(tensor=ps.tensor, offset=ps.offset,
                                    ap=[ps.ap[0], [128, 6], [1, 64]])
                    od_in = bass.AP(tensor=ps.tensor, offset=ps.offset + 64,
                                    ap=[ps.ap[0], [128, 6], [1, 64]])
                    nc.scalar.activation(out=ev_out[:tsz], in_=ev_in[:tsz], func=AF.Copy,
                                         scale=CV / (SH * sv))
                    nc.scalar.activation(out=od_out[:tsz], in_=od_in[:tsz], func=AF.Copy,
                                         scale=CV / (SH * sv))
                return q_c, k_sb, vpe, vpo

            def make_C_units(it, attn):
                """proj subunits, ln2 list, fc1 list, fc2 subunits for item."""
                st = {}

                def proj_u(t, o, tsz, o2, w2, last):
                    def f():
                        if f"r1_{t}" not in st:
                            st[f"r1_{t}"] = work.tile([128, D], F32, name=f"r1_{it}_{t}",
                                                      tag="r1t", bufs=10)
                            xr = work.tile([128, D], F32, name=f"xr_{it}_{t}", tag="xin", bufs=3)
                            t0 = it * N
                            nc.sync.dma_start(out=xr[:tsz, :], in_=x_d[t0 + o:t0 + o + tsz, :])
                            st[f"xr_{t}"] = xr
                        tszw = 66 if tsz == 65 else tsz
                        ps = psum.tile([128, w2], F32, name=f"pspj_{it}_{t}_{o2}",
                                       tag="small", bufs=2)
                        for kcp in KCP:
                            nc.tensor.matmul(ps[:tszw, :],
                                             attn[:, kcp:kcp + 2, o:o + tszw],
                                             wproj_sb[:, kcp:kcp + 2, o2:o2 + w2],
                                             start=(kcp == 0),
                                             stop=(kcp == 4 and not use_bias_mm),
                                             perf_mode=DR)
                        if use_bias_mm:
                            nc.tensor.matmul(ps[:tsz, :], ones_row[0:1, o:o + tsz],
                                             bprojr_sb[0:1, o2:o2 + w2],
                                             start=False, stop=True)
                        nc.vector.scalar_tensor_tensor(
                            out=st[f"r1_{t}"][:tsz, o2:o2 + w2],
                            in0=ps[:tsz], scalar=1.0 / (SA * sp),
                            in1=st[f"xr_{t}"][:tsz, o2:o2 + w2],
                            op0=ALU.mult, op1=ALU.add)
                        if last:
                            # LN2 stats chain early: dump's transposes then
                            # never wait on the DVE chain
                            st[f"ht2_{t}"] = ln_stats(st[f"r1_{t}"][:tsz, :],
                                                      it, t, tsz, 2)
                    return f

                def ln2_t(t, o, tsz):
                    def f():
                        if "h2T" not in st:
                            st["h2T"] = work.tile([128, DCH, NPAD], F8, name=f"h2T_{it}",
                                                  tag="fmbuf", bufs=2)
                            nc.gpsimd.memset(st["h2T"][:, :, N:NPAD], 0.0)
                        ln_finish(st[f"ht2_{t}"], st["h2T"], it, t, tsz, o, 2)
                    return f

                def fc1_mc(mc):
                    def f():
                        if "gelu" not in st:
                            st["gelu"] = work.tile([128, HCH, NPAD], F8, name=f"g_{it}", tag="gelu")
                            nc.gpsimd.memset(st["gelu"][:, :, N:NPAD], 0.0)
                        ps = psum.tile([128, N], F32, name=f"psf1_{it}_{mc}", tag="big", bufs=3)
                        for kcp in KCP:
                            for (o, w) in SPL_N2:
                                nc.tensor.matmul(ps[:, o:o + w],
                                                 wfc1_sb[:, kcp:kcp + 2, mc * 128:(mc + 1) * 128],
                                                 st["h2T"][:, kcp:kcp + 2, o:o + w],
                                                 start=(kcp == 0 and o != 256),
                                                 stop=(kcp == 4),
                                                 perf_mode=DR)
                        nc.scalar.activation(out=st["gelu"][:, mc, 0:N], in_=ps,
                                             func=AF.Gelu,
                                             bias=bfc1_sb[:, mc:mc + 1],
                                             scale=1.0 / (SH * s1))
                    return f

                def fc2_u(t, o, tsz, o2, w2, first, last):
                    def f():
                        t0 = it * N
                        if first:
                            ot = work.tile([128, D], F32, name=f"ot_{it}_{t}", tag="ot", bufs=2)
                            st[f"ot_{t}"] = ot
                        ot = st[f"ot_{t}"]
                        tszw = 66 if tsz == 65 else tsz
                        ps = psum.tile([128, w2], F32, name=f"psf2_{it}_{t}_{o2}",
                                       tag="small", bufs=2)
                        for kcp in KCP2:
                            nc.tensor.matmul(ps[:tszw, :],
                                             st["gelu"][:, kcp:kcp + 2, o:o + tszw],
                                             wfc2_sb[:, kcp:kcp + 2, o2:o2 + w2],
                                             start=(kcp == 0),
                                             stop=(kcp == HCH - 2 and not use_bias_mm),
                                             perf_mode=DR)
                        if use_bias_mm:
                            nc.tensor.matmul(ps[:tsz, :], ones_row[0:1, o:o + tsz],
                                             bfc2r_sb[0:1, o2:o2 + w2],
                                             start=False, stop=True)
                        nc.vector.scalar_tensor_tensor(
                            out=ot[:tsz, o2:o2 + w2],
                            in0=ps[:tsz], scalar=1.0 / s2,
                            in1=st[f"r1_{t}"][:tsz, o2:o2 + w2],
                            op0=ALU.mult, op1=ALU.add)
                        if last:
                            nc.sync.dma_start(out=out_d[t0 + o:t0 + o + tsz, :],
                                              in_=ot[:tsz, :])
                    return f

                gen = []
                for t, (o, tsz) in enumerate(NT):
                    for j, (o2, w2) in enumerate(SPL_D2):
                        gen.append(proj_u(t, o, tsz, o2, w2, j == len(SPL_D2) - 1))
                dump = [ln2_t(t, o, tsz) for t, (o, tsz) in enumerate(NT)]
                dump += [fc1_mc(mc) for mc in range(HCH)]
                for t, (o, tsz) in enumerate(NT):
                    for j, (o2, w2) in enumerate(SPL_D2):
                        gen.append(fc2_u(t, o, tsz, o2, w2, j == 0, j == len(SPL_D2) - 1))
                return gen, dump

            def emit_B(it, q_c, k_sb, vpe, vpo, dump, next_it=None):
                pre_ln_next = []
                """Head-pair loop: row-packed S^T pairs, softmax, PV, norm.

                Fill policy: pairs 0-2 pop proj units of item it-1 at 5 sites
                each; after pair 3 the pending norms are flushed and the whole
                ln2+fc1 block of it-1 is emitted (batched Gelu, warm PE);
                pairs 4-5 and the following emit_A pop fc2 units of it-1.
                """
                attn = work.tile([128, DCH, NPAD], F8, name=f"attn_{it}", tag="attnbuf", bufs=2)
                nc.gpsimd.memset(attn[:, :, N:NPAD], 0.0)
                pend = []

                def flush_norm():
                    if not pend:
                        return
                    pv_e, pv_o, csb2, c = pend.pop(0)
                    # one bc tile per pair: den_e (lane 64) broadcast to rows
                    # 0:64, den_o (lane 0) to rows 64:128 via col tile_position
                    bc = psum.tile([128, N], F32, name=f"psbc_{it}_{c}", tag="big", bufs=3)
                    for (o, w) in SPL_N:
                        nc.tensor.matmul(bc[0:64, o:o + w], ones128[64:65, 0:64],
                                         csb2[64:65, o:o + w], start=True, stop=True)
                        nc.tensor.matmul(bc[64:128, o:o + w], ones128[0:1, 0:64],
                                         csb2[0:1, o:o + w], start=True, stop=True)
                    rec = work.tile([128, N], F32, name=f"rec_{it}_{c}", tag="recbuf", bufs=1)
                    nc.vector.reciprocal_approx_fast(out=rec, in_=bc)
                    nc.vector.tensor_tensor(out=attn[0:64, c, 0:N],
                                            in0=pv_e[0:64, :], in1=rec[0:64, :],
                                            op=ALU.mult)
                    nc.vector.tensor_tensor(out=attn[64:128, c, 0:N],
                                            in0=pv_o[64:128, :], in1=rec[64:128, :],
                                            op=ALU.mult)

                for c in range(DCH):       # head pair (2c, 2c+1)
                    # flush previous pair's norms first: releases its pv PSUM
                    # tiles before this pair's ss allocations rotate the ring
                    flush_norm()
                    flush_norm()
                    es2 = work.tile([128, 5, 2, NPAD], F8, name=f"es_{it}_{c}", tag="ebuf", bufs=2)
                    for mt, (mo, msz) in enumerate(NT):
                        ss_e = psum.tile([128, N], F32, name=f"pse_{it}_{c}_{mt}",
                                         tag="big", bufs=3)
                        for (o, w) in SPL_N:
                            nc.tensor.matmul(ss_e[:, o:o + w],
                                             k_sb[0:64, c, mo:mo + 128],
                                             q_c[0:64, c, o:o + w],
                                             start=True, stop=True)
                        ss_o = psum.tile([128, N], F32, name=f"pso_{it}_{c}_{mt}",
                                         tag="big", bufs=3)
                        for (o, w) in SPL_N:
                            nc.tensor.matmul(ss_o[:, o:o + w],
                                             k_sb[64:128, c, mo:mo + 128],
                                             q_c[64:128, c, o:o + w],
                                             start=True, stop=True)
                        e_t = work.tile([128, N], BF16, name=f"ee_{it}_{c}_{mt}",
                                        tag="etmp", bufs=4)
                        nc.vector.tensor_tensor(out=e_t, in0=ss_e,
                                                in1=maskt_sb[:, mt, :], op=ALU.mult)
                        nc.scalar.activation(out=es2[:, mt, 0, 0:N], in_=e_t, func=AF.Exp)
                        e_t = work.tile([128, N], BF16, name=f"eo_{it}_{c}_{mt}",
                                        tag="etmp", bufs=4)
                        nc.vector.tensor_tensor(out=e_t, in0=ss_o,
                                                in1=maskt_sb[:, mt, :], op=ALU.mult)
                        nc.scalar.activation(out=es2[:, mt, 1, 0:N], in_=e_t, func=AF.Exp)
                        if c < 3 and mt in (1, 2, 3):
                            unit()
                    # PV even head (h=2c): dims rows 0:64, den row 64
                    pv_e = psum.tile([128, N], F32, name=f"pve_{it}_{c}", tag="big", bufs=3)
                    for mtp in (0, 2):
                        for (o, w) in SPL_N2:
                            nc.tensor.matmul(pv_e[0:66, o:o + w],
                                             vpe[:, mtp:mtp + 2, c, 0:66],
                                             es2[:, mtp:mtp + 2, 0, o:o + w],
                                             start=(mtp == 0 and o != 256),
                                             stop=False,
                                             perf_mode=DR)
                    for (o, w) in SPL_N:
                        nc.tensor.matmul(pv_e[0:66, o:o + w], vpe[:, 4, c, 0:66],
                                         es2[:, 4, 0, o:o + w],
                                         start=False, stop=True)
                    csb2 = work.tile([128, N], BF16, name=f"cs_{it}_{c}", tag="csbuf", bufs=2)
                    nc.scalar.activation(out=csb2[64:65, :], in_=pv_e[64:65, :], func=AF.Copy)
                    if c < 3:
                        unit()
                    # PV odd head (h=2c+1): den rows 0:64, dims rows 64:128
                    pv_o = psum.tile([128, N], F32, name=f"pvo_{it}_{c}", tag="big", bufs=3)
                    for mtp in (0, 2):
                        for (o, w) in SPL_N2:
                            nc.tensor.matmul(pv_o[0:128, o:o + w],
                                             vpo[:, mtp:mtp + 2, c, 0:128],
                                             es2[:, mtp:mtp + 2, 1, o:o + w],
                                             start=(mtp == 0 and o != 256),
                                             stop=False,
                                             perf_mode=DR)
                    for (o, w) in SPL_N:
                        nc.tensor.matmul(pv_o[0:128, o:o + w], vpo[:, 4, c, 0:128],
                                         es2[:, 4, 1, o:o + w],
                                         start=False, stop=True)
                    nc.scalar.activation(out=csb2[0:1, :], in_=pv_o[0:1, :], func=AF.Copy)
                    pend.append((pv_e, pv_o, csb2, c))
                    if c < 3:
                        unit()
                    elif c >= 4:
                        tail_unit()
                    if c == 3:
                        flush_norm()
                        flush_norm()
                        for u in dump:
                            u()
                        if next_it is not None:
                            pre_ln_next.extend(ln1_chain(next_it))
                flush_norm()
                flush_norm()
                return attn, pre_ln_next

            dump = []
            pre_ln = ln1_chain(0, pre_x=xts0)
            q_c, k_sb, vpe, vpo = emit_A(0, pre_ln)
            nc.sync.dma_start(out=wproj_sb,
                              in_=wproj_d.rearrange("(c p) m -> p c m", p=128))
            nc.sync.dma_start(out=wfc2_sb,
                              in_=wfc2_d.rearrange("(c p) m -> p c m", p=128))
            nc.sync.dma_start(out=wfc1_sb,
                              in_=wfc1_d.rearrange("(c p) m -> p c m", p=128))
            for it in range(IPC):
                attn, pre_ln = emit_B(it, q_c, k_sb, vpe, vpo, dump,
                                      next_it=it + 1 if it + 1 < IPC else None)
                gen, dump = make_C_units(it, attn)
                queue.extend(gen[:15])
                if it + 1 < IPC:
                    q_c, k_sb, vpe, vpo = emit_A(it + 1, pre_ln)
                # fc2 units of item `it` become poppable only after its
                # ln2+fc1 dump has run (inside emit_B(it+1) / epilogue)
                tailq.extend(gen[15:])
            for u in queue + dump + tailq:
                u()

    nc.compile()
    return nc


def _wscale(w):
    """power-of-2 scale so absmax*scale lands in ~[96, 192]"""
    return float(2.0 ** np.floor(np.log2(192.0 / np.abs(w).max())))


def prep_in_maps(x, cp_mask, ln1_g, ln1_b, w_qkv, w_proj, b_proj,
                 ln2_g, ln2_b, w_fc1, b_fc1, w_fc2, b_fc2):
    bf = ml_dtypes.bfloat16
    f8 = ml_dtypes.float8_e4m3
    f = np.float32
    x = np.asarray(x, f)
    w_qkv = np.asarray(w_qkv, f)
    w_proj = np.asarray(w_proj, f)
    w_fc1 = np.asarray(w_fc1, f)
    w_fc2 = np.asarray(w_fc2, f)
    g1 = np.asarray(ln1_g, f)
    b1 = np.asarray(ln1_b, f)
    g2 = np.asarray(ln2_g, f)
    b2 = np.asarray(ln2_b, f)

    wqkv_eff = w_qkv * g1[:, None]
    bqkv = b1 @ w_qkv
    scale = DH ** -0.5
    wq = np.ascontiguousarray(wqkv_eff[:, 0:D] * scale)
    wk = np.ascontiguousarray(wqkv_eff[:, D:2 * D])
    wv = np.ascontiguousarray(wqkv_eff[:, 2 * D:3 * D])
    wfc1_eff = w_fc1 * g2[:, None]

    sq, sk, sv = _wscale(wq), _wscale(wk), _wscale(wv)
    sp, s1, s2 = _wscale(w_proj), _wscale(wfc1_eff), _wscale(w_fc2)
    scales = (sq, sk, sv, sp, s1, s2)

    # q/k biases pre-scaled so out = (psum + b) * 1/(SH*s)
    bq = (bqkv[0:D] * scale * SH * sq).astype(f)
    bk = (bqkv[D:2 * D] * SH * sk).astype(f)
    bv = bqkv[2 * D:3 * D]

    bprojr = ((np.asarray(b_proj, f) + bv @ w_proj) * SA * sp).astype(bf)
    bfc1_eff = (np.asarray(b_fc1, f) + b2 @ w_fc1).astype(f)
    bfc2r = (np.asarray(b_fc2, f) * s2).astype(bf)

    maskt = np.ascontiguousarray(np.asarray(cp_mask, f)[0, 0].T).astype(bf)
    xs = x.reshape(NCORES, TOK, D)

    shared = dict(maskt=maskt,
                  wq=(wq * sq).astype(f8), wk=(wk * sk).astype(f8),
                  wv=(wv * sv).astype(f8), bq=bq, bk=bk,
                  wproj=(w_proj * sp).astype(f8), bprojr=bprojr,
                  wfc1=(wfc1_eff * s1).astype(f8), bfc1=bfc1_eff,
                  wfc2=(w_fc2 * s2).astype(f8), bfc2r=bfc2r)
    return scales, [dict(x=np.ascontiguousarray(xs[i]), **shared) for i in range(NCORES)]


_NC_CACHE = {}


def get_nc(scales, use_bias_mm=True):
    key = ("nc", scales, use_bias_mm)
    if key not in _NC_CACHE:
        _NC_CACHE[key] = build_nc(scales, use_bias_mm=use_bias_mm)
    return _NC_CACHE[key]


def run(scales, in_maps, trace=False, **kw):
    need_bias = bool(np.any(in_maps[0]["bprojr"].astype(np.float32))
                     or np.any(in_maps[0]["bfc2r"].astype(np.float32)))
    nc = get_nc(scales, use_bias_mm=need_bias)
    if not need_bias:
        in_maps = [{k: v for k, v in m.items() if k not in ("bprojr", "bfc2r")}
                   for m in in_maps]
    return bass_utils.run_bass_kernel_spmd(nc, in_maps, core_ids=list(range(NCORES)),
                                           trace=trace, **kw)


def kernel(**inputs):
    scales, in_maps = prep_in_maps(**inputs)
    res = run(scales, in_maps)
    out = np.stack([res.results[i]["out"] for i in range(NCORES)])
    return out.reshape(B, N, D).astype(np.float32)


# revision 16
# speedup vs baseline: 1.0057x; 1.0057x over previous
"""Trainium2 Bass kernel for a ViT-style transformer block (sparse_attention).

Strategy: data-parallel over batch B=32 across 8 cores (4 items/core), no
collectives. v4 over v3: fp8e4m3 DoubleRow matmuls (2 contraction rows per
cycle) for qkv/v, PV, proj, fc1, fc2 — S^T stays bf16 with head-pair row
packing. All quantization scales are powers of two folded into existing
ops: the q/k bias tensor_scalar gains a mult, v/hT evacuation copies gain an
ACTIVATE scale, gelu descales via its scale operand, proj/fc2 evacuation
uses scalar_tensor_tensor (descale+residual in one DVE op), and the
PV ones-column value CV/SA makes the softmax normalization come out in
attn-scale SA directly. fp8 weights halve SBUF pressure: wfc1 is resident
(no per-item fetches) and r1 stays in SBUF (no DRAM round-trip).

Host-side folding: LN gammas into the following weight matrices, LN betas
into qkv/fc1 biases, v-bias into the proj bias, attention scale into w_q,
power-of-2 fp8 ranges into all weights.
"""

import sys

sys.path.insert(0, "/opt/trn_rl_repo")

import numpy as np
import ml_dtypes

import concourse.bass as bass
import concourse.tile as tile
from concourse import bacc, mybir
from concourse import bass_utils
from concourse.masks import make_identity



F32 = mybir.dt.float32
BF16 = mybir.dt.bfloat16
F8 = mybir.dt.float8e4
DR = mybir.MatmulPerfMode.DoubleRow

B = 32
N = 577
D = 768
H = 12
DH = 64
HID = 3072
DCH = D // 128          # 6 chunks of the model dim
HCH = HID // 128        # 24 chunks of the hidden dim
NCORES = 8
IPC = B // NCORES       # items per core
TOK = IPC * N           # tokens per core

NT = [(0, 128), (128, 128), (256, 128), (384, 128), (512, 65)]
NPAD = 592              # fp8 feature-major tiles padded: dual-fp8 LDWEIGHTS
                        # requires even (aligned) k-subtile pair strides
EPS = 1e-5
SH = 16.0               # hT / h2T fp8 store scale
CV = 8.0                # v fp8 store scale
SA = 16.0               # attn fp8 store scale


def _nsplits(total, cap=512):
    out = []
    o = 0
    while o < total:
        w = min(cap, total - o)
        out.append((o, w))
        o += w
    return out


SPL_N = _nsplits(N)             # bf16 moving chunks
SPL_N2 = _nsplits(N, 256)       # DoubleRow moving chunks (free = 2w <= 512)
SPL_D2 = _nsplits(D, 256)
KCP = [0, 2, 4]                 # kc pair starts over D contraction
KCP2 = list(range(0, HCH, 2))   # over HID contraction
AF = mybir.ActivationFunctionType
ALU = mybir.AluOpType


def build_nc(scales, use_bias_mm=True):
    sq, sk, sv, sp, s1, s2 = scales
    nc = bacc.Bacc("TRN2", target_bir_lowering=False, debug=False, num_devices=NCORES)

    x_d = nc.dram_tensor("x", [TOK, D], F32, kind="ExternalInput").ap()
    maskt_d = nc.dram_tensor("maskt", [N, N], BF16, kind="ExternalInput").ap()
    wq_d = nc.dram_tensor("wq", [D, D], F8, kind="ExternalInput").ap()
    wk_d = nc.dram_tensor("wk", [D, D], F8, kind="ExternalInput").ap()
    wv_d = nc.dram_tensor("wv", [D, D], F8, kind="ExternalInput").ap()
    bq_d = nc.dram_tensor("bq", [D], F32, kind="ExternalInput").ap()
    bk_d = nc.dram_tensor("bk", [D], F32, kind="ExternalInput").ap()
    wproj_d = nc.dram_tensor("wproj", [D, D], F8, kind="ExternalInput").ap()
    wfc1_d = nc.dram_tensor("wfc1", [D, HID], F8, kind="ExternalInput").ap()
    bfc1_d = nc.dram_tensor("bfc1", [HID], F32, kind="ExternalInput").ap()
    wfc2_d = nc.dram_tensor("wfc2", [HID, D], F8, kind="ExternalInput").ap()
    if use_bias_mm:
        bprojr_d = nc.dram_tensor("bprojr", [D], BF16, kind="ExternalInput").ap()
        bfc2r_d = nc.dram_tensor("bfc2r", [D], BF16, kind="ExternalInput").ap()
    out_d = nc.dram_tensor("out", [TOK, D], F32, kind="ExternalOutput").ap()

    with tile.TileContext(nc) as tc:
        with (
            tc.tile_pool(name="const", bufs=1) as const,
            tc.tile_pool(name="work", bufs=1) as work,
            tc.tile_pool(name="psum", bufs=1, space="PSUM") as psum,
        ):
            # ---- item-0 x tiles first so LN1 can start immediately ----
            xts0 = []
            for t, (o, tsz) in enumerate(NT):
                # own 5-deep ring: all 5 DMAs are issued before any consumer
                # exists, so a shorter ring would overwrite live tiles
                xt = work.tile([128, D], F32, name=f"xin_0_{t}", tag="xin0", bufs=5)
                nc.sync.dma_start(out=xt[:tsz, :], in_=x_d[o:o + tsz, :])
                xts0.append(xt)

            # ---- constants / weights (resident) ----
            wq_sb = const.tile([128, DCH, D], F8, name="wq_sb")
            wq_r = wq_d.rearrange("(c p) m -> p c m", p=128)
            for kcp in KCP:
                nc.sync.dma_start(out=wq_sb[:, kcp:kcp + 2, :], in_=wq_r[:, kcp:kcp + 2, :])
            bq_sb = const.tile([128, DCH], F32, name="bq_sb")
            nc.sync.dma_start(out=bq_sb, in_=bq_d.rearrange("(c p) -> p c", p=128))
            wk_sb = const.tile([128, DCH, D], F8, name="wk_sb")
            wk_r = wk_d.rearrange("(c p) m -> p c m", p=128)
            for kcp in KCP:
                nc.sync.dma_start(out=wk_sb[:, kcp:kcp + 2, :], in_=wk_r[:, kcp:kcp + 2, :])
            bk_sb = const.tile([128, DCH], F32, name="bk_sb")
            nc.sync.dma_start(out=bk_sb, in_=bk_d.rearrange("(c p) -> p c", p=128))
            wv_sb = const.tile([128, DCH, D], F8, name="wv_sb")
            nc.sync.dma_start(out=wv_sb, in_=wv_d.rearrange("(c p) m -> p c m", p=128))

            maskt_sb = const.tile([128, 5, N], BF16, name="maskt_sb")
            nc.gpsimd.memset(maskt_sb[:, 4, :], 0.0)
            for mt, (mo, msz) in enumerate(NT):
                nc.sync.dma_start(out=maskt_sb[:msz, mt, :], in_=maskt_d[mo:mo + msz, :])

            wproj_sb = const.tile([128, DCH, D], F8, name="wproj_sb")
            wfc1_sb = const.tile([128, DCH, HID], F8, name="wfc1_sb")
            wfc2_sb = const.tile([128, HCH, D], F8, name="wfc2_sb")

            bfc1_sb = const.tile([128, HCH], F32, name="bfc1_sb")
            nc.sync.dma_start(out=bfc1_sb, in_=bfc1_d.rearrange("(c p) -> p c", p=128))
            if use_bias_mm:
                bprojr_sb = const.tile([1, D], BF16, name="bprojr_sb")
                nc.sync.dma_start(out=bprojr_sb, in_=bprojr_d[None, :])
                bfc2r_sb = const.tile([1, D], BF16, name="bfc2r_sb")
                nc.sync.dma_start(out=bfc2r_sb, in_=bfc2r_d[None, :])
                ones_row = const.tile([1, N], BF16, name="ones_row")
                nc.vector.memset(ones_row, 1.0)

            ident = const.tile([128, 128], BF16, name="ident")
            make_identity(nc, ident)
            ones128 = const.tile([128, 128], BF16, name="ones128")
            nc.gpsimd.memset(ones128, 1.0)

            eps_sb = const.tile([128, 1], F32, name="eps_sb")
            nc.vector.memset(eps_sb, EPS)

            # ---- fill-work queues (units of item i-1) ----
            queue = []      # proj units: popped during head-pairs 0-2
            tailq = []      # fc2 units: popped during head-pairs 4-5 + emit_A

            def unit(n=1):
                for _ in range(n):
                    if queue:
                        queue.pop(0)()

            def tail_unit(n=1):
                for _ in range(n):
                    if tailq:
                        tailq.pop(0)()

            def ln_stats(src, it, t, tsz, ph):
                """LN(src[tsz, D]) -> ht_tm bf16 (DVE/Scalar chain only)."""
                stats = work.tile([128, 2, 6], F32, name=f"st{ph}_{it}_{t}", tag="stats", bufs=3)
                nc.vector.bn_stats(out=stats[:tsz, 0, :], in_=src[:, 0:512])
                nc.vector.bn_stats(out=stats[:tsz, 1, :], in_=src[:, 512:768])
                mv = work.tile([128, 2], F32, name=f"mv{ph}_{it}_{t}", tag="mv", bufs=6)
                nc.vector.bn_aggr(out=mv[:tsz], in_=stats[:tsz])
                rstd = work.tile([128, 1], F32, name=f"rs{ph}_{it}_{t}", tag="rstd", bufs=6)
                nc.scalar.activation(out=rstd[:tsz], in_=mv[:tsz, 1:2],
                                     func=AF.Sqrt, bias=eps_sb[:tsz], scale=1.0)
                nc.vector.reciprocal(out=rstd[:tsz], in_=rstd[:tsz])
                ht_tm = work.tile([128, D], BF16, name=f"htm{ph}_{it}_{t}", tag="htm", bufs=12)
                nc.vector.tensor_scalar(out=ht_tm[:tsz], in0=src,
                                        scalar1=mv[:tsz, 0:1], scalar2=rstd[:tsz],
                                        op0=ALU.subtract, op1=ALU.mult)
                return ht_tm

            def ln_finish(ht_tm, dst_fm, it, t, tsz, o, ph):
                """PE transpose of ht_tm -> fp8*SH -> dst_fm[:, :, o:o+tsz]."""
                tp_ps = psum.tile([128, D], BF16, name=f"tp{ph}_{it}_{t}", tag="small", bufs=2)
                for c in range(DCH):
                    nc.tensor.transpose(tp_ps[:, c * 128:c * 128 + tsz],
                                        ht_tm[:tsz, c * 128:(c + 1) * 128],
                                        ident[:tsz, :tsz])
                nc.scalar.activation(out=dst_fm[:, :, o:o + tsz],
                                     in_=tp_ps.rearrange("p (c q) -> p c q", c=DCH)[:, :, :tsz],
                                     func=AF.Copy, scale=SH)

            def ln1_chain(it, pre_x=None):
                """x loads + LN1 stats chains for all 5 tiles of item `it`."""
                hts = []
                for t, (o, tsz) in enumerate(NT):
                    if pre_x is not None:
                        xt = pre_x[t]
                    else:
                        xt = work.tile([128, D], F32, name=f"xin_{it}_{t}", tag="xin0", bufs=5)
                        nc.sync.dma_start(out=xt[:tsz, :],
                                          in_=x_d[it * N + o:it * N + o + tsz, :])
                    hts.append(ln_stats(xt[:tsz, :], it, t, tsz, 1))
                return hts

            def layernorm_tp(src, dst_fm, it, t, tsz, o, ph):
                ln_finish(ln_stats(src, it, t, tsz, ph), dst_fm, it, t, tsz, o, ph)

            def emit_A(it, pre_ln):
                """LN1 transposes (from precomputed ht_tms), q/k/v, vplus."""
                hT = work.tile([128, DCH, NPAD], F8, name=f"hT_{it}", tag="fmbuf", bufs=2)
                nc.gpsimd.memset(hT[:, :, N:NPAD], 0.0)
                for t, (o, tsz) in enumerate(NT):
                    # pop fill units BEFORE each transpose block so their
                    # (ready) PE work precedes the chain-dependent transposes
                    tail_unit(3)
                    ln_finish(pre_ln[t], hT, it, t, tsz, o, 1)

                # q_c: packed q, chunk c rows 0:64 = head 2c, rows 64:128 = head 2c+1
                q_c = work.tile([128, DCH, N], BF16, name=f"q_{it}", tag="qbuf")
                # k padded to 640 cols; cols 577:640 zero (tail-tile K padding)
                k_sb = work.tile([128, DCH, 640], BF16, name=f"k_{it}", tag="kbuf")
                nc.gpsimd.memset(k_sb[:, :, 577:640], 0.0)
                for mc in range(DCH):
                    ps = psum.tile([128, N], F32, name=f"psq_{it}_{mc}", tag="big", bufs=3)
                    for kcp in KCP:
                        for (o, w) in SPL_N2:
                            nc.tensor.matmul(ps[:, o:o + w],
                                             wq_sb[:, kcp:kcp + 2, mc * 128:(mc + 1) * 128],
                                             hT[:, kcp:kcp + 2, o:o + w],
                                             start=(kcp == 0 and o != 256),
                                             stop=(kcp == 4),
                                             perf_mode=DR)
                    nc.vector.tensor_scalar(out=q_c[:, mc, :], in0=ps,
                                            scalar1=bq_sb[:, mc:mc + 1],
                                            scalar2=1.0 / (SH * sq),
                                            op0=ALU.add, op1=ALU.mult)
                for mc in range(DCH):
                    ps = psum.tile([128, N], F32, name=f"psk_{it}_{mc}", tag="big", bufs=3)
                    for kcp in KCP:
                        for (o, w) in SPL_N2:
                            nc.tensor.matmul(ps[:, o:o + w],
                                             wk_sb[:, kcp:kcp + 2, mc * 128:(mc + 1) * 128],
                                             hT[:, kcp:kcp + 2, o:o + w],
                                             start=(kcp == 0 and o != 256),
                                             stop=(kcp == 4),
                                             perf_mode=DR)
                    nc.vector.tensor_scalar(out=k_sb[:, mc, 0:N], in0=ps,
                                            scalar1=bk_sb[:, mc:mc + 1],
                                            scalar2=1.0 / (SH * sk),
                                            op0=ALU.add, op1=ALU.mult)

                # vpe: [dims(64) | ones] per (mt, c) for even heads;
                # vpo: [ones(64) | dims(64)] for odd heads. ones value CV/SA.
                vpe = work.tile([128, 5, DCH, 80], F8, name=f"vpe_{it}", tag="vpe")
                vpo = work.tile([128, 5, DCH, 128], F8, name=f"vpo_{it}", tag="vpo")
                for t, (o, tsz) in enumerate(NT):
                    if tsz < 128:
                        for vp in (vpe, vpo):
                            nc.gpsimd.memset(vp[64:128, t], 0.0)
                            nc.gpsimd.memset(vp[0:tsz, t], CV / SA)
                    else:
                        nc.gpsimd.memset(vpe[:, t, :, 64:65], CV / SA)
                        nc.gpsimd.memset(vpo[:, t, :, 0:64], CV / SA)
                    nc.gpsimd.memset(vpe[:, t, :, 65:66], 0.0)
                    tszw = 66 if tsz == 65 else tsz
                    ps = psum.tile([128, D], F32, name=f"psv_{it}_{t}", tag="big", bufs=3)
                    for kcp in KCP:
                        for (o2, w2) in SPL_D2:
                            nc.tensor.matmul(ps[:tszw, o2:o2 + w2],
                                             hT[:, kcp:kcp + 2, o:o + tszw],
                                             wv_sb[:, kcp:kcp + 2, o2:o2 + w2],
                                             start=(kcp == 0 and o2 != 256),
                                             stop=(kcp == 4),
                                             perf_mode=DR)
                    toff = t * DCH * 80
                    ev_out = bass.AP(tensor=vpe.tensor, offset=vpe.offset + toff,
                                     ap=[vpe.ap[0], [80, 6], [1, 64]])
                    toff = t * DCH * 128
                    od_out = bass.AP(tensor=vpo.tensor, offset=vpo.offset + toff + 64,
                                     ap=[vpo.ap[0], [128, 6], [1, 64]])
                    ev_in = bass.AP(tensor=ps.tensor, offset=ps.offset,
                                    ap=[ps.ap[0], [128, 6], [1, 64]])
                    od_in = bass.AP(tensor=ps.tensor, offset=ps.offset + 64,
                                    ap=[ps.ap[0], [128, 6], [1, 64]])
                    nc.scalar.activation(out=ev_out[:tsz], in_=ev_in[:tsz], func=AF.Copy,
                                         scale=CV / (SH * sv))
                    nc.scalar.activation(out=od_out[:tsz], in_=od_in[:tsz], func=AF.Copy,
                                         scale=CV / (SH * sv))
                return q_c, k_sb, vpe, vpo

            def make_C_units(it, attn):
                """proj subunits, ln2 list, fc1 list, fc2 subunits for item."""
                st = {}

                def proj_u(t, o, tsz, o2, w2, last):
                    def f():
                        if f"r1_{t}" not in st:
                            st[f"r1_{t}"] = work.tile([128, D], F32, name=f"r1_{it}_{t}",
                                                      tag="r1t", bufs=10)
                            xr = work.tile([128, D], F32, name=f"xr_{it}_{t}", tag="xin", bufs=3)
                            t0 = it * N
                            nc.sync.dma_start(out=xr[:tsz, :], in_=x_d[t0 + o:t0 + o + tsz, :])
                            st[f"xr_{t}"] = xr
                        tszw = 66 if tsz == 65 else tsz
                        ps = psum.tile([128, w2], F32, name=f"pspj_{it}_{t}_{o2}",
                                       tag="small", bufs=2)
                        for kcp in KCP:
                            nc.tensor.matmul(ps[:tszw, :],
                                             attn[:, kcp:kcp + 2, o:o + tszw],
                                             wproj_sb[:, kcp:kcp + 2, o2:o2 + w2],
                                             start=(kcp == 0),
                                             stop=(kcp == 4 and not use_bias_mm),
                                             perf_mode=DR)
                        if use_bias_mm:
                            nc.tensor.matmul(ps[:tsz, :], ones_row[0:1, o:o + tsz],
                                             bprojr_sb[0:1, o2:o2 + w2],
                                             start=False, stop=True)
                        nc.vector.scalar_tensor_tensor(
                            out=st[f"r1_{t}"][:tsz, o2:o2 + w2],
                            in0=ps[:tsz], scalar=1.0 / (SA * sp),
                            in1=st[f"xr_{t}"][:tsz, o2:o2 + w2],
                            op0=ALU.mult, op1=ALU.add)
                    return f

                def ln2_t(t, o, tsz):
                    def f():
                        if "h2T" not in st:
                            st["h2T"] = work.tile([128, DCH, NPAD], F8, name=f"h2T_{it}",
                                                  tag="fmbuf", bufs=2)
                            nc.gpsimd.memset(st["h2T"][:, :, N:NPAD], 0.0)
                        layernorm_tp(st[f"r1_{t}"][:tsz, :], st["h2T"], it, t, tsz, o, 2)
                    return f

                def fc1_mc(mc):
                    def f():
                        if "gelu" not in st:
                            st["gelu"] = work.tile([128, HCH, NPAD], F8, name=f"g_{it}", tag="gelu")
                            nc.gpsimd.memset(st["gelu"][:, :, N:NPAD], 0.0)
                        ps = psum.tile([128, N], F32, name=f"psf1_{it}_{mc}", tag="big", bufs=3)
                        for kcp in KCP:
                            for (o, w) in SPL_N2:
                                nc.tensor.matmul(ps[:, o:o + w],
                                                 wfc1_sb[:, kcp:kcp + 2, mc * 128:(mc + 1) * 128],
                                                 st["h2T"][:, kcp:kcp + 2, o:o + w],
                                                 start=(kcp == 0 and o != 256),
                                                 stop=(kcp == 4),
                                                 perf_mode=DR)
                        nc.scalar.activation(out=st["gelu"][:, mc, 0:N], in_=ps,
                                             func=AF.Gelu,
                                             bias=bfc1_sb[:, mc:mc + 1],
                                             scale=1.0 / (SH * s1))
                    return f

                def fc2_u(t, o, tsz, o2, w2, first, last):
                    def f():
                        t0 = it * N
                        if first:
                            ot = work.tile([128, D], F32, name=f"ot_{it}_{t}", tag="ot", bufs=2)
                            st[f"ot_{t}"] = ot
                        ot = st[f"ot_{t}"]
                        tszw = 66 if tsz == 65 else tsz
                        ps = psum.tile([128, w2], F32, name=f"psf2_{it}_{t}_{o2}",
                                       tag="small", bufs=2)
                        for kcp in KCP2:
                            nc.tensor.matmul(ps[:tszw, :],
                                             st["gelu"][:, kcp:kcp + 2, o:o + tszw],
                                             wfc2_sb[:, kcp:kcp + 2, o2:o2 + w2],
                                             start=(kcp == 0),
                                             stop=(kcp == HCH - 2 and not use_bias_mm),
                                             perf_mode=DR)
                        if use_bias_mm:
                            nc.tensor.matmul(ps[:tsz, :], ones_row[0:1, o:o + tsz],
                                             bfc2r_sb[0:1, o2:o2 + w2],
                                             start=False, stop=True)
                        nc.vector.scalar_tensor_tensor(
                            out=ot[:tsz, o2:o2 + w2],
                            in0=ps[:tsz], scalar=1.0 / s2,
                            in1=st[f"r1_{t}"][:tsz, o2:o2 + w2],
                            op0=ALU.mult, op1=ALU.add)
                        if last:
                            nc.sync.dma_start(out=out_d[t0 + o:t0 + o + tsz, :],
                                              in_=ot[:tsz, :])
                    return f

                gen = []
                for t, (o, tsz) in enumerate(NT):
                    for j, (o2, w2) in enumerate(SPL_D2):
                        gen.append(proj_u(t, o, tsz, o2, w2, j == len(SPL_D2) - 1))
                dump = [ln2_t(t, o, tsz) for t, (o, tsz) in enumerate(NT)]
                dump += [fc1_mc(mc) for mc in range(HCH)]
                for t, (o, tsz) in enumerate(NT):
                    for j, (o2, w2) in enumerate(SPL_D2):
                        gen.append(fc2_u(t, o, tsz, o2, w2, j == 0, j == len(SPL_D2) - 1))
                return gen, dump

            def emit_B(it, q_c, k_sb, vpe, vpo, dump, next_it=None):
                pre_ln_next = []
                """Head-pair loop: row-packed S^T pairs, softmax, PV, norm.

                Fill policy: pairs 0-2 pop proj units of item it-1 at 5 sites
                each; after pair 3 the pending norms are flushed and the whole
                ln2+fc1 block of it-1 is emitted (batched Gelu, warm PE);
                pairs 4-5 and the following emit_A pop fc2 units of it-1.
                """
                attn = work.tile([128, DCH, NPAD], F8, name=f"attn_{it}", tag="attnbuf", bufs=2)
                nc.gpsimd.memset(attn[:, :, N:NPAD], 0.0)
                pend = []

                def flush_norm():
                    if not pend:
                        return
                    pv_e, pv_o, csb2, c = pend.pop(0)
                    # one bc tile per pair: den_e (lane 64) broadcast to rows
                    # 0:64, den_o (lane 0) to rows 64:128 via col tile_position
                    bc = psum.tile([128, N], F32, name=f"psbc_{it}_{c}", tag="big", bufs=3)
                    for (o, w) in SPL_N:
                        nc.tensor.matmul(bc[0:64, o:o + w], ones128[64:65, 0:64],
                                         csb2[64:65, o:o + w], start=True, stop=True)
                        nc.tensor.matmul(bc[64:128, o:o + w], ones128[0:1, 0:64],
                                         csb2[0:1, o:o + w], start=True, stop=True)
                    rec = work.tile([128, N], F32, name=f"rec_{it}_{c}", tag="recbuf", bufs=1)
                    nc.vector.reciprocal_approx_fast(out=rec, in_=bc)
                    nc.vector.tensor_tensor(out=attn[0:64, c, 0:N],
                                            in0=pv_e[0:64, :], in1=rec[0:64, :],
                                            op=ALU.mult)
                    nc.vector.tensor_tensor(out=attn[64:128, c, 0:N],
                                            in0=pv_o[64:128, :], in1=rec[64:128, :],
                                            op=ALU.mult)

                for c in range(DCH):       # head pair (2c, 2c+1)
                    # flush previous pair's norms first: releases its pv PSUM
                    # tiles before this pair's ss allocations rotate the ring
                    flush_norm()
                    flush_norm()
                    es2 = work.tile([128, 5, 2, NPAD], F8, name=f"es_{it}_{c}", tag="ebuf", bufs=2)
                    for mt, (mo, msz) in enumerate(NT):
                        ss_e = psum.tile([128, N], F32, name=f"pse_{it}_{c}_{mt}",
                                         tag="big", bufs=3)
                        for (o, w) in SPL_N:
                            nc.tensor.matmul(ss_e[:, o:o + w],
                                             k_sb[0:64, c, mo:mo + 128],
                                             q_c[0:64, c, o:o + w],
                                             start=True, stop=True)
                        ss_o = psum.tile([128, N], F32, name=f"pso_{it}_{c}_{mt}",
                                         tag="big", bufs=3)
                        for (o, w) in SPL_N:
                            nc.tensor.matmul(ss_o[:, o:o + w],
                                             k_sb[64:128, c, mo:mo + 128],
                                             q_c[64:128, c, o:o + w],
                                             start=True, stop=True)
                        e_t = work.tile([128, N], BF16, name=f"ee_{it}_{c}_{mt}",
                                        tag="etmp", bufs=4)
                        nc.vector.tensor_tensor(out=e_t, in0=ss_e,
                                                in1=maskt_sb[:, mt, :], op=ALU.mult)
                        nc.scalar.activation(out=es2[:, mt, 0, 0:N], in_=e_t, func=AF.Exp)
                        e_t = work.tile([128, N], BF16, name=f"eo_{it}_{c}_{mt}",
                                        tag="etmp", bufs=4)
                        nc.vector.tensor_tensor(out=e_t, in0=ss_o,
                                                in1=maskt_sb[:, mt, :], op=ALU.mult)
                        nc.scalar.activation(out=es2[:, mt, 1, 0:N], in_=e_t, func=AF.Exp)
                        if c < 3 and mt in (1, 2, 3):
                            unit()
                    # PV even head (h=2c): dims rows 0:64, den row 64
                    pv_e = psum.tile([128, N], F32, name=f"pve_{it}_{c}", tag="big", bufs=3)
                    for mtp in (0, 2):
                        for (o, w) in SPL_N2:
                            nc.tensor.matmul(pv_e[0:66, o:o + w],
                                             vpe[:, mtp:mtp + 2, c, 0:66],
                                             es2[:, mtp:mtp + 2, 0, o:o + w],
                                             start=(mtp == 0 and o != 256),
                                             stop=False,
                                             perf_mode=DR)
                    for (o, w) in SPL_N:
                        nc.tensor.matmul(pv_e[0:66, o:o + w], vpe[:, 4, c, 0:66],
                                         es2[:, 4, 0, o:o + w],
                                         start=False, stop=True)
                    csb2 = work.tile([128, N], BF16, name=f"cs_{it}_{c}", tag="csbuf", bufs=2)
                    nc.scalar.activation(out=csb2[64:65, :], in_=pv_e[64:65, :], func=AF.Copy)
                    if c < 3:
                        unit()
                    # PV odd head (h=2c+1): den rows 0:64, dims rows 64:128
                    pv_o = psum.tile([128, N], F32, name=f"pvo_{it}_{c}", tag="big", bufs=3)
                    for mtp in (0, 2):
                        for (o, w) in SPL_N2:
                            nc.tensor.matmul(pv_o[0:128, o:o + w],
                                             vpo[:, mtp:mtp + 2, c, 0:128],
                                             es2[:, mtp:mtp + 2, 1, o:o + w],
                                             start=(mtp == 0 and o != 256),
                                             stop=False,
                                             perf_mode=DR)
                    for (o, w) in SPL_N:
                        nc.tensor.matmul(pv_o[0:128, o:o + w], vpo[:, 4, c, 0:128],
                                         es2[:, 4, 1, o:o + w],
                                         start=False, stop=True)
                    nc.scalar.activation(out=csb2[0:1, :], in_=pv_o[0:1, :], func=AF.Copy)
                    pend.append((pv_e, pv_o, csb2, c))
                    if c < 3:
                        unit()
                    elif c >= 4:
                        tail_unit()
                    if c == 3:
                        flush_norm()
                        flush_norm()
                        for u in dump:
                            u()
                        if next_it is not None:
                            pre_ln_next.extend(ln1_chain(next_it))
                flush_norm()
                flush_norm()
                return attn, pre_ln_next

            dump = []
            pre_ln = ln1_chain(0, pre_x=xts0)
            q_c, k_sb, vpe, vpo = emit_A(0, pre_ln)
            nc.sync.dma_start(out=wproj_sb,
                              in_=wproj_d.rearrange("(c p) m -> p c m", p=128))
            nc.sync.dma_start(out=wfc2_sb,
                              in_=wfc2_d.rearrange("(c p) m -> p c m", p=128))
            nc.sync.dma_start(out=wfc1_sb,
                              in_=wfc1_d.rearrange("(c p) m -> p c m", p=128))
            for it in range(IPC):
                attn, pre_ln = emit_B(it, q_c, k_sb, vpe, vpo, dump,
                                      next_it=it + 1 if it + 1 < IPC else None)
                gen, dump = make_C_units(it, attn)
                queue.extend(gen[:15])
                if it + 1 < IPC:
                    q_c, k_sb, vpe, vpo = emit_A(it + 1, pre_ln)
                # fc2 units of item `it` become poppable only after its
                # ln2+fc1 dump has run (inside emit_B(it+1) / epilogue)
                tailq.extend(gen[15:])
            for u in queue + dump + tailq:
                u()

    nc.compile()
    return nc


def _wscale(w):
    """power-of-2 scale so absmax*scale lands in ~[96, 192]"""
    return float(2.0 ** np.floor(np.log2(192.0 / np.abs(w).max())))


def prep_in_maps(x, cp_mask, ln1_g, ln1_b, w_qkv, w_proj, b_proj,
                 ln2_g, ln2_b, w_fc1, b_fc1, w_fc2, b_fc2):
    bf = ml_dtypes.bfloat16
    f8 = ml_dtypes.float8_e4m3
    f = np.float32
    x = np.asarray(x, f)
    w_qkv = np.asarray(w_qkv, f)
    w_proj = np.asarray(w_proj, f)
    w_fc1 = np.asarray(w_fc1, f)
    w_fc2 = np.asarray(w_fc2, f)
    g1 = np.asarray(ln1_g, f)
    b1 = np.asarray(ln1_b, f)
    g2 = np.asarray(ln2_g, f)
    b2 = np.asarray(ln2_b, f)

    wqkv_eff = w_qkv * g1[:, None]
    bqkv = b1 @ w_qkv
    scale = DH ** -0.5
    wq = np.ascontiguousarray(wqkv_eff[:, 0:D] * scale)
    wk = np.ascontiguousarray(wqkv_eff[:, D:2 * D])
    wv = np.ascontiguousarray(wqkv_eff[:, 2 * D:3 * D])
    wfc1_eff = w_fc1 * g2[:, None]

    sq, sk, sv = _wscale(wq), _wscale(wk), _wscale(wv)
    sp, s1, s2 = _wscale(w_proj), _wscale(wfc1_eff), _wscale(w_fc2)
    scales = (sq, sk, sv, sp, s1, s2)

    # q/k biases pre-scaled so out = (psum + b) * 1/(SH*s)
    bq = (bqkv[0:D] * scale * SH * sq).astype(f)
    bk = (bqkv[D:2 * D] * SH * sk).astype(f)
    bv = bqkv[2 * D:3 * D]

    bprojr = ((np.asarray(b_proj, f) + bv @ w_proj) * SA * sp).astype(bf)
    bfc1_eff = (np.asarray(b_fc1, f) + b2 @ w_fc1).astype(f)
    bfc2r = (np.asarray(b_fc2, f) * s2).astype(bf)

    maskt = np.ascontiguousarray(np.asarray(cp_mask, f)[0, 0].T).astype(bf)
    xs = x.reshape(NCORES, TOK, D)

    shared = dict(maskt=maskt,
                  wq=(wq * sq).astype(f8), wk=(wk * sk).astype(f8),
                  wv=(wv * sv).astype(f8), bq=bq, bk=bk,
                  wproj=(w_proj * sp).astype(f8), bprojr=bprojr,
                  wfc1=(wfc1_eff * s1).astype(f8), bfc1=bfc1_eff,
                  wfc2=(w_fc2 * s2).astype(f8), bfc2r=bfc2r)
    return scales, [dict(x=np.ascontiguousarray(xs[i]), **shared) for i in range(NCORES)]


_NC_CACHE = {}


def get_nc(scales, use_bias_mm=True):
    key = ("nc", scales, use_bias_mm)
    if key not in _NC_CACHE:
        _NC_CACHE[key] = build_nc(scales, use_bias_mm=use_bias_mm)
    return _NC_CACHE[key]


def run(scales, in_maps, trace=False, **kw):
    need_bias = bool(np.any(in_maps[0]["bprojr"].astype(np.float32))
                     or np.any(in_maps[0]["bfc2r"].astype(np.float32)))
    nc = get_nc(scales, use_bias_mm=need_bias)
    if not need_bias:
        in_maps = [{k: v for k, v in m.items() if k not in ("bprojr", "bfc2r")}
                   for m in in_maps]
    return bass_utils.run_bass_kernel_spmd(nc, in_maps, core_ids=list(range(NCORES)),
                                           trace=trace, **kw)


def kernel(**inputs):
    scales, in_maps = prep_in_maps(**inputs)
    res = run(scales, in_maps)
    out = np.stack([res.results[i]["out"] for i in range(NCORES)])
    return out.reshape(B, N, D).astype(np.float32)


# revision 18
# speedup vs baseline: 1.0221x; 1.0163x over previous
"""Trainium2 Bass kernel for a ViT-style transformer block (sparse_attention).

Strategy: data-parallel over batch B=32 across 8 cores (4 items/core), no
collectives. v4 over v3: fp8e4m3 DoubleRow matmuls (2 contraction rows per
cycle) for qkv/v, PV, proj, fc1, fc2 — S^T stays bf16 with head-pair row
packing. All quantization scales are powers of two folded into existing
ops: the q/k bias tensor_scalar gains a mult, v/hT evacuation copies gain an
ACTIVATE scale, gelu descales via its scale operand, proj/fc2 evacuation
uses scalar_tensor_tensor (descale+residual in one DVE op), and the
PV ones-column value CV/SA makes the softmax normalization come out in
attn-scale SA directly. fp8 weights halve SBUF pressure: wfc1 is resident
(no per-item fetches) and r1 stays in SBUF (no DRAM round-trip).

Host-side folding: LN gammas into the following weight matrices, LN betas
into qkv/fc1 biases, v-bias into the proj bias, attention scale into w_q,
power-of-2 fp8 ranges into all weights.
"""

import sys

sys.path.insert(0, "/opt/trn_rl_repo")

import numpy as np
import ml_dtypes

import concourse.bass as bass
import concourse.tile as tile
from concourse import bacc, mybir
from concourse import bass_utils
from concourse.masks import make_identity



F32 = mybir.dt.float32
BF16 = mybir.dt.bfloat16
F8 = mybir.dt.float8e4
DR = mybir.MatmulPerfMode.DoubleRow

B = 32
N = 577
D = 768
H = 12
DH = 64
HID = 3072
DCH = D // 128          # 6 chunks of the model dim
HCH = HID // 128        # 24 chunks of the hidden dim
NCORES = 8
IPC = B // NCORES       # items per core
TOK = IPC * N           # tokens per core

NT = [(0, 128), (128, 128), (256, 128), (384, 128), (512, 65)]
NPAD = 592              # fp8 feature-major tiles padded: dual-fp8 LDWEIGHTS
                        # requires even (aligned) k-subtile pair strides
EPS = 1e-5
SH = 16.0               # hT / h2T fp8 store scale
CV = 8.0                # v fp8 store scale
SA = 16.0               # attn fp8 store scale


def _nsplits(total, cap=512):
    out = []
    o = 0
    while o < total:
        w = min(cap, total - o)
        out.append((o, w))
        o += w
    return out


SPL_N = _nsplits(N)             # bf16 moving chunks
SPL_N2 = _nsplits(N, 256)       # DoubleRow moving chunks (free = 2w <= 512)
SPL_D2 = _nsplits(D, 256)
KCP = [0, 2, 4]                 # kc pair starts over D contraction
KCP2 = list(range(0, HCH, 2))   # over HID contraction
AF = mybir.ActivationFunctionType
ALU = mybir.AluOpType


def build_nc(scales, use_bias_mm=True):
    sq, sk, sv, sp, s1, s2 = scales
    nc = bacc.Bacc("TRN2", target_bir_lowering=False, debug=False, num_devices=NCORES)

    x_d = nc.dram_tensor("x", [TOK, D], F32, kind="ExternalInput").ap()
    maskt_d = nc.dram_tensor("maskt", [N, N], BF16, kind="ExternalInput").ap()
    wq_d = nc.dram_tensor("wq", [D, D], F8, kind="ExternalInput").ap()
    wk_d = nc.dram_tensor("wk", [D, D], F8, kind="ExternalInput").ap()
    wv_d = nc.dram_tensor("wv", [D, D], F8, kind="ExternalInput").ap()
    bq_d = nc.dram_tensor("bq", [D], F32, kind="ExternalInput").ap()
    bk_d = nc.dram_tensor("bk", [D], F32, kind="ExternalInput").ap()
    wproj_d = nc.dram_tensor("wproj", [D, D], F8, kind="ExternalInput").ap()
    wfc1_d = nc.dram_tensor("wfc1", [D, HID], F8, kind="ExternalInput").ap()
    bfc1_d = nc.dram_tensor("bfc1", [HID], F32, kind="ExternalInput").ap()
    wfc2_d = nc.dram_tensor("wfc2", [HID, D], F8, kind="ExternalInput").ap()
    if use_bias_mm:
        bprojr_d = nc.dram_tensor("bprojr", [D], BF16, kind="ExternalInput").ap()
        bfc2r_d = nc.dram_tensor("bfc2r", [D], BF16, kind="ExternalInput").ap()
    out_d = nc.dram_tensor("out", [TOK, D], F32, kind="ExternalOutput").ap()

    with tile.TileContext(nc) as tc:
        with (
            tc.tile_pool(name="const", bufs=1) as const,
            tc.tile_pool(name="work", bufs=1) as work,
            tc.tile_pool(name="psum", bufs=1, space="PSUM") as psum,
        ):
            # ---- item-0 x tiles first so LN1 can start immediately ----
            xts0 = []
            for t, (o, tsz) in enumerate(NT):
                # own 5-deep ring: all 5 DMAs are issued before any consumer
                # exists, so a shorter ring would overwrite live tiles
                xt = work.tile([128, D], F32, name=f"xin_0_{t}", tag="xin0", bufs=5)
                nc.sync.dma_start(out=xt[:tsz, 0:512], in_=x_d[o:o + tsz, 0:512])
                nc.sync.dma_start(out=xt[:tsz, 512:D], in_=x_d[o:o + tsz, 512:D])
                xts0.append(xt)

            # ---- constants / weights (resident) ----
            wq_sb = const.tile([128, DCH, D], F8, name="wq_sb")
            wq_r = wq_d.rearrange("(c p) m -> p c m", p=128)
            for kcp in KCP:
                nc.sync.dma_start(out=wq_sb[:, kcp:kcp + 2, :], in_=wq_r[:, kcp:kcp + 2, :])
            bq_sb = const.tile([128, DCH], F32, name="bq_sb")
            nc.sync.dma_start(out=bq_sb, in_=bq_d.rearrange("(c p) -> p c", p=128))
            wk_sb = const.tile([128, DCH, D], F8, name="wk_sb")
            wk_r = wk_d.rearrange("(c p) m -> p c m", p=128)
            for kcp in KCP:
                nc.sync.dma_start(out=wk_sb[:, kcp:kcp + 2, :], in_=wk_r[:, kcp:kcp + 2, :])
            bk_sb = const.tile([128, DCH], F32, name="bk_sb")
            nc.sync.dma_start(out=bk_sb, in_=bk_d.rearrange("(c p) -> p c", p=128))
            wv_sb = const.tile([128, DCH, D], F8, name="wv_sb")
            nc.sync.dma_start(out=wv_sb, in_=wv_d.rearrange("(c p) m -> p c m", p=128))

            maskt_sb = const.tile([128, 5, N], BF16, name="maskt_sb")
            nc.gpsimd.memset(maskt_sb[:, 4, :], 0.0)
            for mt, (mo, msz) in enumerate(NT):
                nc.sync.dma_start(out=maskt_sb[:msz, mt, :], in_=maskt_d[mo:mo + msz, :])

            wproj_sb = const.tile([128, DCH, D], F8, name="wproj_sb")
            wfc1_sb = const.tile([128, DCH, HID], F8, name="wfc1_sb")
            wfc2_sb = const.tile([128, HCH, D], F8, name="wfc2_sb")

            bfc1_sb = const.tile([128, HCH], F32, name="bfc1_sb")
            nc.sync.dma_start(out=bfc1_sb, in_=bfc1_d.rearrange("(c p) -> p c", p=128))
            if use_bias_mm:
                bprojr_sb = const.tile([1, D], BF16, name="bprojr_sb")
                nc.sync.dma_start(out=bprojr_sb, in_=bprojr_d[None, :])
                bfc2r_sb = const.tile([1, D], BF16, name="bfc2r_sb")
                nc.sync.dma_start(out=bfc2r_sb, in_=bfc2r_d[None, :])
                ones_row = const.tile([1, N], BF16, name="ones_row")
                nc.vector.memset(ones_row, 1.0)

            ident = const.tile([128, 128], BF16, name="ident")
            make_identity(nc, ident)
            ones128 = const.tile([128, 128], BF16, name="ones128")
            nc.gpsimd.memset(ones128, 1.0)

            eps_sb = const.tile([128, 1], F32, name="eps_sb")
            nc.vector.memset(eps_sb, EPS)

            # ---- fill-work queues (units of item i-1) ----
            queue = []      # proj units: popped during head-pairs 0-2
            tailq = []      # fc2 units: popped during head-pairs 4-5 + emit_A

            def unit(n=1):
                for _ in range(n):
                    if queue:
                        queue.pop(0)()

            def tail_unit(n=1):
                for _ in range(n):
                    if tailq:
                        tailq.pop(0)()

            def ln_stats(src, it, t, tsz, ph):
                """LN(src[tsz, D]) -> ht_tm bf16 (DVE/Scalar chain only)."""
                stats = work.tile([128, 2, 6], F32, name=f"st{ph}_{it}_{t}", tag="stats", bufs=3)
                nc.vector.bn_stats(out=stats[:tsz, 0, :], in_=src[:, 0:512])
                nc.vector.bn_stats(out=stats[:tsz, 1, :], in_=src[:, 512:768])
                mv = work.tile([128, 2], F32, name=f"mv{ph}_{it}_{t}", tag="mv", bufs=6)
                nc.vector.bn_aggr(out=mv[:tsz], in_=stats[:tsz])
                rstd = work.tile([128, 1], F32, name=f"rs{ph}_{it}_{t}", tag="rstd", bufs=6)
                nc.scalar.activation(out=rstd[:tsz], in_=mv[:tsz, 1:2],
                                     func=AF.Sqrt, bias=eps_sb[:tsz], scale=1.0)
                nc.vector.reciprocal(out=rstd[:tsz], in_=rstd[:tsz])
                ht_tm = work.tile([128, D], BF16, name=f"htm{ph}_{it}_{t}", tag="htm", bufs=12)
                nc.vector.tensor_scalar(out=ht_tm[:tsz], in0=src,
                                        scalar1=mv[:tsz, 0:1], scalar2=rstd[:tsz],
                                        op0=ALU.subtract, op1=ALU.mult)
                return ht_tm

            def ln_finish(ht_tm, dst_fm, it, t, tsz, o, ph):
                """PE transpose of ht_tm -> fp8*SH -> dst_fm[:, :, o:o+tsz]."""
                tp_ps = psum.tile([128, D], BF16, name=f"tp{ph}_{it}_{t}", tag="small", bufs=2)
                for c in range(DCH):
                    nc.tensor.transpose(tp_ps[:, c * 128:c * 128 + tsz],
                                        ht_tm[:tsz, c * 128:(c + 1) * 128],
                                        ident[:tsz, :tsz])
                nc.scalar.activation(out=dst_fm[:, :, o:o + tsz],
                                     in_=tp_ps.rearrange("p (c q) -> p c q", c=DCH)[:, :, :tsz],
                                     func=AF.Copy, scale=SH)

            def ln1_chain(it, pre_x=None):
                """x loads + LN1 stats chains for all 5 tiles of item `it`."""
                hts = []
                for t, (o, tsz) in enumerate(NT):
                    if pre_x is not None:
                        xt = pre_x[t]
                    else:
                        xt = work.tile([128, D], F32, name=f"xin_{it}_{t}", tag="xin0", bufs=5)
                        nc.sync.dma_start(out=xt[:tsz, :],
                                          in_=x_d[it * N + o:it * N + o + tsz, :])
                    hts.append(ln_stats(xt[:tsz, :], it, t, tsz, 1))
                return hts

            def layernorm_tp(src, dst_fm, it, t, tsz, o, ph):
                ln_finish(ln_stats(src, it, t, tsz, ph), dst_fm, it, t, tsz, o, ph)

            def emit_A(it, pre_ln):
                """LN1 transposes (from precomputed ht_tms), q/k/v, vplus."""
                hT = work.tile([128, DCH, NPAD], F8, name=f"hT_{it}", tag="fmbuf", bufs=2)
                nc.gpsimd.memset(hT[:, :, N:NPAD], 0.0)
                for t, (o, tsz) in enumerate(NT):
                    # pop fill units BEFORE each transpose block so their
                    # (ready) PE work precedes the chain-dependent transposes
                    tail_unit(3)
                    ln_finish(pre_ln[t], hT, it, t, tsz, o, 1)

                # q_c: packed q, chunk c rows 0:64 = head 2c, rows 64:128 = head 2c+1
                q_c = work.tile([128, DCH, N], BF16, name=f"q_{it}", tag="qbuf")
                # k padded to 640 cols; cols 577:640 zero (tail-tile K padding)
                k_sb = work.tile([128, DCH, 640], BF16, name=f"k_{it}", tag="kbuf")
                nc.gpsimd.memset(k_sb[:, :, 577:640], 0.0)
                for mc in range(DCH):
                    ps = psum.tile([128, N], F32, name=f"psq_{it}_{mc}", tag="big", bufs=3)
                    for kcp in KCP:
                        for (o, w) in SPL_N2:
                            nc.tensor.matmul(ps[:, o:o + w],
                                             wq_sb[:, kcp:kcp + 2, mc * 128:(mc + 1) * 128],
                                             hT[:, kcp:kcp + 2, o:o + w],
                                             start=(kcp == 0 and o != 256),
                                             stop=(kcp == 4),
                                             perf_mode=DR)
                    nc.vector.tensor_scalar(out=q_c[:, mc, :], in0=ps,
                                            scalar1=bq_sb[:, mc:mc + 1],
                                            scalar2=1.0 / (SH * sq),
                                            op0=ALU.add, op1=ALU.mult)
                for mc in range(DCH):
                    ps = psum.tile([128, N], F32, name=f"psk_{it}_{mc}", tag="big", bufs=3)
                    for kcp in KCP:
                        for (o, w) in SPL_N2:
                            nc.tensor.matmul(ps[:, o:o + w],
                                             wk_sb[:, kcp:kcp + 2, mc * 128:(mc + 1) * 128],
                                             hT[:, kcp:kcp + 2, o:o + w],
                                             start=(kcp == 0 and o != 256),
                                             stop=(kcp == 4),
                                             perf_mode=DR)
                    nc.vector.tensor_scalar(out=k_sb[:, mc, 0:N], in0=ps,
                                            scalar1=bk_sb[:, mc:mc + 1],
                                            scalar2=1.0 / (SH * sk),
                                            op0=ALU.add, op1=ALU.mult)

                # vpe: [dims(64) | ones] per (mt, c) for even heads;
                # vpo: [ones(64) | dims(64)] for odd heads. ones value CV/SA.
                vpe = work.tile([128, 5, DCH, 80], F8, name=f"vpe_{it}", tag="vpe")
                vpo = work.tile([128, 5, DCH, 128], F8, name=f"vpo_{it}", tag="vpo")
                for t, (o, tsz) in enumerate(NT):
                    if tsz < 128:
                        for vp in (vpe, vpo):
                            nc.gpsimd.memset(vp[64:128, t], 0.0)
                            nc.gpsimd.memset(vp[0:tsz, t], CV / SA)
                    else:
                        nc.gpsimd.memset(vpe[:, t, :, 64:65], CV / SA)
                        nc.gpsimd.memset(vpo[:, t, :, 0:64], CV / SA)
                    nc.gpsimd.memset(vpe[:, t, :, 65:66], 0.0)
                    tszw = 66 if tsz == 65 else tsz
                    ps = psum.tile([128, D], F32, name=f"psv_{it}_{t}", tag="big", bufs=3)
                    for kcp in KCP:
                        for (o2, w2) in SPL_D2:
                            nc.tensor.matmul(ps[:tszw, o2:o2 + w2],
                                             hT[:, kcp:kcp + 2, o:o + tszw],
                                             wv_sb[:, kcp:kcp + 2, o2:o2 + w2],
                                             start=(kcp == 0 and o2 != 256),
                                             stop=(kcp == 4),
                                             perf_mode=DR)
                    toff = t * DCH * 80
                    ev_out = bass.AP(tensor=vpe.tensor, offset=vpe.offset + toff,
                                     ap=[vpe.ap[0], [80, 6], [1, 64]])
                    toff = t * DCH * 128
                    od_out = bass.AP(tensor=vpo.tensor, offset=vpo.offset + toff + 64,
                                     ap=[vpo.ap[0], [128, 6], [1, 64]])
                    ev_in = bass.AP(tensor=ps.tensor, offset=ps.offset,
                                    ap=[ps.ap[0], [128, 6], [1, 64]])
                    od_in = bass.AP(tensor=ps.tensor, offset=ps.offset + 64,
                                    ap=[ps.ap[0], [128, 6], [1, 64]])
                    nc.scalar.activation(out=ev_out[:tsz], in_=ev_in[:tsz], func=AF.Copy,
                                         scale=CV / (SH * sv))
                    nc.scalar.activation(out=od_out[:tsz], in_=od_in[:tsz], func=AF.Copy,
                                         scale=CV / (SH * sv))
                return q_c, k_sb, vpe, vpo

            def make_C_units(it, attn):
                """proj subunits, ln2 list, fc1 list, fc2 subunits for item."""
                st = {}
                stats_early = (it == IPC - 1)

                def proj_u(t, o, tsz, o2, w2, last):
                    def f():
                        if f"r1_{t}" not in st:
                            st[f"r1_{t}"] = work.tile([128, D], F32, name=f"r1_{it}_{t}",
                                                      tag="r1t", bufs=10)
                            xr = work.tile([128, D], F32, name=f"xr_{it}_{t}", tag="xin", bufs=3)
                            t0 = it * N
                            nc.sync.dma_start(out=xr[:tsz, :], in_=x_d[t0 + o:t0 + o + tsz, :])
                            st[f"xr_{t}"] = xr
                        tszw = 66 if tsz == 65 else tsz
                        ps = psum.tile([128, w2], F32, name=f"pspj_{it}_{t}_{o2}",
                                       tag="small", bufs=2)
                        for kcp in KCP:
                            nc.tensor.matmul(ps[:tszw, :],
                                             attn[:, kcp:kcp + 2, o:o + tszw],
                                             wproj_sb[:, kcp:kcp + 2, o2:o2 + w2],
                                             start=(kcp == 0),
                                             stop=(kcp == 4 and not use_bias_mm),
                                             perf_mode=DR)
                        if use_bias_mm:
                            nc.tensor.matmul(ps[:tsz, :], ones_row[0:1, o:o + tsz],
                                             bprojr_sb[0:1, o2:o2 + w2],
                                             start=False, stop=True)
                        nc.vector.scalar_tensor_tensor(
                            out=st[f"r1_{t}"][:tsz, o2:o2 + w2],
                            in0=ps[:tsz], scalar=1.0 / (SA * sp),
                            in1=st[f"xr_{t}"][:tsz, o2:o2 + w2],
                            op0=ALU.mult, op1=ALU.add)
                        if last and stats_early:
                            st[f"ht2_{t}"] = ln_stats(st[f"r1_{t}"][:tsz, :],
                                                      it, t, tsz, 2)
                    return f

                def ln2_t(t, o, tsz):
                    def f():
                        if "h2T" not in st:
                            st["h2T"] = work.tile([128, DCH, NPAD], F8, name=f"h2T_{it}",
                                                  tag="fmbuf", bufs=2)
                            nc.gpsimd.memset(st["h2T"][:, :, N:NPAD], 0.0)
                        if f"ht2_{t}" in st:
                            ln_finish(st[f"ht2_{t}"], st["h2T"], it, t, tsz, o, 2)
                        else:
                            layernorm_tp(st[f"r1_{t}"][:tsz, :], st["h2T"],
                                         it, t, tsz, o, 2)
                    return f

                def fc1_mc(mc):
                    def f():
                        if "gelu" not in st:
                            st["gelu"] = work.tile([128, HCH, NPAD], F8, name=f"g_{it}", tag="gelu")
                            nc.gpsimd.memset(st["gelu"][:, :, N:NPAD], 0.0)
                        ps = psum.tile([128, N], F32, name=f"psf1_{it}_{mc}", tag="big", bufs=3)
                        for kcp in KCP:
                            for (o, w) in SPL_N2:
                                nc.tensor.matmul(ps[:, o:o + w],
                                                 wfc1_sb[:, kcp:kcp + 2, mc * 128:(mc + 1) * 128],
                                                 st["h2T"][:, kcp:kcp + 2, o:o + w],
                                                 start=(kcp == 0 and o != 256),
                                                 stop=(kcp == 4),
                                                 perf_mode=DR)
                        nc.scalar.activation(out=st["gelu"][:, mc, 0:N], in_=ps,
                                             func=AF.Gelu,
                                             bias=bfc1_sb[:, mc:mc + 1],
                                             scale=1.0 / (SH * s1))
                    return f

                def fc2_u(t, o, tsz, o2, w2, first, last):
                    def f():
                        t0 = it * N
                        if first:
                            ot = work.tile([128, D], F32, name=f"ot_{it}_{t}", tag="ot", bufs=2)
                            st[f"ot_{t}"] = ot
                        ot = st[f"ot_{t}"]
                        tszw = 66 if tsz == 65 else tsz
                        ps = psum.tile([128, w2], F32, name=f"psf2_{it}_{t}_{o2}",
                                       tag="small", bufs=2)
                        for kcp in KCP2:
                            nc.tensor.matmul(ps[:tszw, :],
                                             st["gelu"][:, kcp:kcp + 2, o:o + tszw],
                                             wfc2_sb[:, kcp:kcp + 2, o2:o2 + w2],
                                             start=(kcp == 0),
                                             stop=(kcp == HCH - 2 and not use_bias_mm),
                                             perf_mode=DR)
                        if use_bias_mm:
                            nc.tensor.matmul(ps[:tsz, :], ones_row[0:1, o:o + tsz],
                                             bfc2r_sb[0:1, o2:o2 + w2],
                                             start=False, stop=True)
                        nc.vector.scalar_tensor_tensor(
                            out=ot[:tsz, o2:o2 + w2],
                            in0=ps[:tsz], scalar=1.0 / s2,
                            in1=st[f"r1_{t}"][:tsz, o2:o2 + w2],
                            op0=ALU.mult, op1=ALU.add)
                        if last:
                            nc.sync.dma_start(out=out_d[t0 + o:t0 + o + tsz, :],
                                              in_=ot[:tsz, :])
                    return f

                gen = []
                for t, (o, tsz) in enumerate(NT):
                    for j, (o2, w2) in enumerate(SPL_D2):
                        gen.append(proj_u(t, o, tsz, o2, w2, j == len(SPL_D2) - 1))
                dump = [ln2_t(t, o, tsz) for t, (o, tsz) in enumerate(NT)]
                dump += [fc1_mc(mc) for mc in range(HCH)]
                for t, (o, tsz) in enumerate(NT):
                    for j, (o2, w2) in enumerate(SPL_D2):
                        gen.append(fc2_u(t, o, tsz, o2, w2, j == 0, j == len(SPL_D2) - 1))
                return gen, dump

            def emit_B(it, q_c, k_sb, vpe, vpo, dump, next_it=None):
                pre_ln_next = []
                """Head-pair loop: row-packed S^T pairs, softmax, PV, norm.

                Fill policy: pairs 0-2 pop proj units of item it-1 at 5 sites
                each; after pair 3 the pending norms are flushed and the whole
                ln2+fc1 block of it-1 is emitted (batched Gelu, warm PE);
                pairs 4-5 and the following emit_A pop fc2 units of it-1.
                """
                attn = work.tile([128, DCH, NPAD], F8, name=f"attn_{it}", tag="attnbuf", bufs=2)
                nc.gpsimd.memset(attn[:, :, N:NPAD], 0.0)
                pend = []

                def flush_norm():
                    if not pend:
                        return
                    pv_e, pv_o, csb2, c = pend.pop(0)
                    # one bc tile per pair: den_e (lane 64) broadcast to rows
                    # 0:64, den_o (lane 0) to rows 64:128 via col tile_position
                    bc = psum.tile([128, N], F32, name=f"psbc_{it}_{c}", tag="big", bufs=3)
                    for (o, w) in SPL_N:
                        nc.tensor.matmul(bc[0:64, o:o + w], ones128[64:65, 0:64],
                                         csb2[64:65, o:o + w], start=True, stop=True)
                        nc.tensor.matmul(bc[64:128, o:o + w], ones128[0:1, 0:64],
                                         csb2[0:1, o:o + w], start=True, stop=True)
                    rec = work.tile([128, N], F32, name=f"rec_{it}_{c}", tag="recbuf", bufs=1)
                    nc.vector.reciprocal_approx_fast(out=rec, in_=bc)
                    nc.vector.tensor_tensor(out=attn[0:64, c, 0:N],
                                            in0=pv_e[0:64, :], in1=rec[0:64, :],
                                            op=ALU.mult)
                    nc.vector.tensor_tensor(out=attn[64:128, c, 0:N],
                                            in0=pv_o[64:128, :], in1=rec[64:128, :],
                                            op=ALU.mult)

                for c in range(DCH):       # head pair (2c, 2c+1)
                    # flush previous pair's norms first: releases its pv PSUM
                    # tiles before this pair's ss allocations rotate the ring
                    flush_norm()
                    flush_norm()
                    es2 = work.tile([128, 5, 2, NPAD], F8, name=f"es_{it}_{c}", tag="ebuf", bufs=2)
                    for mt, (mo, msz) in enumerate(NT):
                        ss_e = psum.tile([128, N], F32, name=f"pse_{it}_{c}_{mt}",
                                         tag="big", bufs=3)
                        for (o, w) in SPL_N:
                            nc.tensor.matmul(ss_e[:, o:o + w],
                                             k_sb[0:64, c, mo:mo + 128],
                                             q_c[0:64, c, o:o + w],
                                             start=True, stop=True)
                        ss_o = psum.tile([128, N], F32, name=f"pso_{it}_{c}_{mt}",
                                         tag="big", bufs=3)
                        for (o, w) in SPL_N:
                            nc.tensor.matmul(ss_o[:, o:o + w],
                                             k_sb[64:128, c, mo:mo + 128],
                                             q_c[64:128, c, o:o + w],
                                             start=True, stop=True)
                        e_t = work.tile([128, N], BF16, name=f"ee_{it}_{c}_{mt}",
                                        tag="etmp", bufs=4)
                        nc.vector.tensor_tensor(out=e_t, in0=ss_e,
                                                in1=maskt_sb[:, mt, :], op=ALU.mult)
                        nc.scalar.activation(out=es2[:, mt, 0, 0:N], in_=e_t, func=AF.Exp)
                        e_t = work.tile([128, N], BF16, name=f"eo_{it}_{c}_{mt}",
                                        tag="etmp", bufs=4)
                        nc.vector.tensor_tensor(out=e_t, in0=ss_o,
                                                in1=maskt_sb[:, mt, :], op=ALU.mult)
                        nc.scalar.activation(out=es2[:, mt, 1, 0:N], in_=e_t, func=AF.Exp)
                        if c < 3 and mt in (1, 2, 3):
                            unit()
                    # PV even head (h=2c): dims rows 0:64, den row 64
                    pv_e = psum.tile([128, N], F32, name=f"pve_{it}_{c}", tag="big", bufs=3)
                    for mtp in (0, 2):
                        for (o, w) in SPL_N2:
                            nc.tensor.matmul(pv_e[0:66, o:o + w],
                                             vpe[:, mtp:mtp + 2, c, 0:66],
                                             es2[:, mtp:mtp + 2, 0, o:o + w],
                                             start=(mtp == 0 and o != 256),
                                             stop=False,
                                             perf_mode=DR)
                    for (o, w) in SPL_N:
                        nc.tensor.matmul(pv_e[0:66, o:o + w], vpe[:, 4, c, 0:66],
                                         es2[:, 4, 0, o:o + w],
                                         start=False, stop=True)
                    csb2 = work.tile([128, N], BF16, name=f"cs_{it}_{c}", tag="csbuf", bufs=2)
                    nc.scalar.activation(out=csb2[64:65, :], in_=pv_e[64:65, :], func=AF.Copy)
                    if c < 3:
                        unit()
                    # PV odd head (h=2c+1): den rows 0:64, dims rows 64:128
                    pv_o = psum.tile([128, N], F32, name=f"pvo_{it}_{c}", tag="big", bufs=3)
                    for mtp in (0, 2):
                        for (o, w) in SPL_N2:
                            nc.tensor.matmul(pv_o[0:128, o:o + w],
                                             vpo[:, mtp:mtp + 2, c, 0:128],
                                             es2[:, mtp:mtp + 2, 1, o:o + w],
                                             start=(mtp == 0 and o != 256),
                                             stop=False,
                                             perf_mode=DR)
                    for (o, w) in SPL_N:
                        nc.tensor.matmul(pv_o[0:128, o:o + w], vpo[:, 4, c, 0:128],
                                         es2[:, 4, 1, o:o + w],
                                         start=False, stop=True)
                    nc.scalar.activation(out=csb2[0:1, :], in_=pv_o[0:1, :], func=AF.Copy)
                    pend.append((pv_e, pv_o, csb2, c))
                    if c < 3:
                        unit()
                    elif c >= 4:
                        tail_unit()
                    if c == 3:
                        flush_norm()
                        flush_norm()
                        for u in dump[:5]:
                            u()
                        if next_it is not None:
                            pre_ln_next.extend(ln1_chain(next_it))
                        for u in dump[5:]:
                            u()
                flush_norm()
                flush_norm()
                return attn, pre_ln_next

            dump = []
            pre_ln = ln1_chain(0, pre_x=xts0)
            q_c, k_sb, vpe, vpo = emit_A(0, pre_ln)
            nc.sync.dma_start(out=wproj_sb,
                              in_=wproj_d.rearrange("(c p) m -> p c m", p=128))
            nc.sync.dma_start(out=wfc2_sb,
                              in_=wfc2_d.rearrange("(c p) m -> p c m", p=128))
            nc.sync.dma_start(out=wfc1_sb,
                              in_=wfc1_d.rearrange("(c p) m -> p c m", p=128))
            for it in range(IPC):
                attn, pre_ln = emit_B(it, q_c, k_sb, vpe, vpo, dump,
                                      next_it=it + 1 if it + 1 < IPC else None)
                gen, dump = make_C_units(it, attn)
                queue.extend(gen[:15])
                if it + 1 < IPC:
                    q_c, k_sb, vpe, vpo = emit_A(it + 1, pre_ln)
                # fc2 units of item `it` become poppable only after its
                # ln2+fc1 dump has run (inside emit_B(it+1) / epilogue)
                tailq.extend(gen[15:])
            for u in queue + dump + tailq:
                u()

    nc.compile()
    return nc


def _wscale(w):
    """power-of-2 scale so absmax*scale lands in ~[96, 192]"""
    return float(2.0 ** np.floor(np.log2(192.0 / np.abs(w).max())))


def prep_in_maps(x, cp_mask, ln1_g, ln1_b, w_qkv, w_proj, b_proj,
                 ln2_g, ln2_b, w_fc1, b_fc1, w_fc2, b_fc2):
    bf = ml_dtypes.bfloat16
    f8 = ml_dtypes.float8_e4m3
    f = np.float32
    x = np.asarray(x, f)
    w_qkv = np.asarray(w_qkv, f)
    w_proj = np.asarray(w_proj, f)
    w_fc1 = np.asarray(w_fc1, f)
    w_fc2 = np.asarray(w_fc2, f)
    g1 = np.asarray(ln1_g, f)
    b1 = np.asarray(ln1_b, f)
    g2 = np.asarray(ln2_g, f)
    b2 = np.asarray(ln2_b, f)

    wqkv_eff = w_qkv * g1[:, None]
    bqkv = b1 @ w_qkv
    scale = DH ** -0.5
    wq = np.ascontiguousarray(wqkv_eff[:, 0:D] * scale)
    wk = np.ascontiguousarray(wqkv_eff[:, D:2 * D])
    wv = np.ascontiguousarray(wqkv_eff[:, 2 * D:3 * D])
    wfc1_eff = w_fc1 * g2[:, None]

    sq, sk, sv = _wscale(wq), _wscale(wk), _wscale(wv)
    sp, s1, s2 = _wscale(w_proj), _wscale(wfc1_eff), _wscale(w_fc2)
    scales = (sq, sk, sv, sp, s1, s2)

    # q/k biases pre-scaled so out = (psum + b) * 1/(SH*s)
    bq = (bqkv[0:D] * scale * SH * sq).astype(f)
    bk = (bqkv[D:2 * D] * SH * sk).astype(f)
    bv = bqkv[2 * D:3 * D]

    bprojr = ((np.asarray(b_proj, f) + bv @ w_proj) * SA * sp).astype(bf)
    bfc1_eff = (np.asarray(b_fc1, f) + b2 @ w_fc1).astype(f)
    bfc2r = (np.asarray(b_fc2, f) * s2).astype(bf)

    maskt = np.ascontiguousarray(np.asarray(cp_mask, f)[0, 0].T).astype(bf)
    xs = x.reshape(NCORES, TOK, D)

    shared = dict(maskt=maskt,
                  wq=(wq * sq).astype(f8), wk=(wk * sk).astype(f8),
                  wv=(wv * sv).astype(f8), bq=bq, bk=bk,
                  wproj=(w_proj * sp).astype(f8), bprojr=bprojr,
                  wfc1=(wfc1_eff * s1).astype(f8), bfc1=bfc1_eff,
                  wfc2=(w_fc2 * s2).astype(f8), bfc2r=bfc2r)
    return scales, [dict(x=np.ascontiguousarray(xs[i]), **shared) for i in range(NCORES)]


_NC_CACHE = {}


def get_nc(scales, use_bias_mm=True):
    key = ("nc", scales, use_bias_mm)
    if key not in _NC_CACHE:
        _NC_CACHE[key] = build_nc(scales, use_bias_mm=use_bias_mm)
    return _NC_CACHE[key]


def run(scales, in_maps, trace=False, **kw):
    need_bias = bool(np.any(in_maps[0]["bprojr"].astype(np.float32))
                     or np.any(in_maps[0]["bfc2r"].astype(np.float32)))
    nc = get_nc(scales, use_bias_mm=need_bias)
    if not need_bias:
        in_maps = [{k: v for k, v in m.items() if k not in ("bprojr", "bfc2r")}
                   for m in in_maps]
    return bass_utils.run_bass_kernel_spmd(nc, in_maps, core_ids=list(range(NCORES)),
                                           trace=trace, **kw)


def kernel(**inputs):
    scales, in_maps = prep_in_maps(**inputs)
    res = run(scales, in_maps)
    out = np.stack([res.results[i]["out"] for i in range(NCORES)])
    return out.reshape(B, N, D).astype(np.float32)
